# revision 9
# baseline (speedup 1.0000x reference)
"""Causal self-attention on 8 Trainium2 NeuronCores.

Sharding (batch + head parallel): core c handles batch b = c // 4 and the
4 heads [hg*4, hg*4+4) where hg = c % 4.  Each core computes q/k/v from
column-sliced c_attn weights, full causal attention for its heads, and a
partial c_proj output from the matching row slice of w_proj; the host sums
the 4 partials per batch.

All matmuls run in bf16 (fp32 PSUM accumulate).  The two heads of a pair
are computed CONCURRENTLY in the PE array for the S = K^T Q matmuls via
row tiling (K=64 contraction each, tile_position rows 0-63 / 64-127).
The causal mask is applied inside PSUM with a -30000*tril matmul
accumulate, so the S -> exp -> PV chain only touches PE and ACT.
PV consumption is emitted LAG blocks behind S production so the PE queue
never stalls at head-of-line waiting for the ACT exp (keeps the HAM
clock-gate at 2.4 GHz).
"""

import sys

if "/opt/trn_rl_repo" not in sys.path:
    sys.path.insert(0, "/opt/trn_rl_repo")

import numpy as np

import concourse.mybir as mybir
from concourse import bacc
from concourse.bass_utils import run_bass_kernel_spmd
from concourse.tile import TileContext

B, T, C = 2, 2048, 1024
H, D = 16, 64
HL = 4  # heads per core
N_CORES = 8
KT = C // 128  # contraction tiles over the embedding dim
SCALE = 1.0 / 8.0  # 1/sqrt(D)
NEG = -30000.0
LAG = 2  # PV emission lag (in key blocks) behind S/exp

_CACHE = {}


def _build():
    f32 = mybir.dt.float32
    f32r = mybir.dt.float32r
    bf16 = mybir.dt.bfloat16
    nc = bacc.Bacc("TRN2", target_bir_lowering=False, debug=False, num_devices=N_CORES)

    x_in = nc.dram_tensor("x_in", [128, KT, T], bf16, kind="ExternalInput")
    wqk = nc.dram_tensor("wqk", [128, KT, 2 * HL * D], bf16, kind="ExternalInput")
    wv = nc.dram_tensor("wv", [128, KT, HL * D], bf16, kind="ExternalInput")
    wp = nc.dram_tensor("wp", [128, HL // 2, C], bf16, kind="ExternalInput")
    # consts: cols 0:128 = diag(-30000); cols 128:256 = tril(ones, -1)
    consts = nc.dram_tensor("consts", [128, 256], bf16, kind="ExternalInput")
    ones64 = nc.dram_tensor("ones64", [128, 64], f32r, kind="ExternalInput")
    out = nc.dram_tensor("out", [T, C], bf16, kind="ExternalOutput")

    EXP = mybir.ActivationFunctionType.Exp

    with TileContext(nc) as tc:
        with tc.tile_pool(name="persist", bufs=1) as persist:
            # q/k feature-major [d, t]: slot 0/1 = q head-pairs 0/1, 2/3 = k;
            # partitions 0-63 = even head dims, 64-127 = odd head dims.
            qk_t = [
                [persist.tile([128, 512], bf16, name=f"qk{s}_{tb}") for tb in range(4)]
                for s in range(4)
            ]
            # v token-major per 128-token tile; col D holds ones (denominator)
            v_t = [
                persist.tile([128, HL, D + 1], bf16, name=f"v{tt}") for tt in range(16)
            ]
            # head-pair stacked normalized y per 512-token block
            y2_t = [
                persist.tile([128, HL // 2, 512], bf16, name=f"y2{b_}")
                for b_ in range(4)
            ]
            wp_sb = persist.tile([128, HL // 2, C], bf16)
            nc.sync.dma_start(wp_sb, wp[:, :, :])
            consts_sb = persist.tile([128, 256], bf16)
            nc.sync.dma_start(consts_sb, consts[:, :])
            negI = consts_sb[:, 0:128]
            triM = consts_sb[:, 128:256]
            ones_sb = persist.tile([128, 64], f32r)
            nc.sync.dma_start(ones_sb, ones64[:, :])
            # denominator staging rows: all partitions zero except row D, so
            # the broadcast matmul can contract K=128 (full-array mode, no
            # tiling-mode switch/drain on the PE)
            rsb_slots = [
                persist.tile([128, 2, 512], f32r, name=f"rsb{i}") for i in range(2)
            ]
            for r_ in rsb_slots:
                nc.gpsimd.memset(r_.bitcast(mybir.dt.uint32), 0)

            for tt in range(16):
                nc.gpsimd.memset(v_t[tt][:, :, D : D + 1], 1.0)

            with (
                tc.tile_pool(name="qkvp", bufs=1) as qkvp,
                tc.tile_pool(name="attp", bufs=6) as attp,
                tc.tile_pool(name="attsmall", bufs=2) as attsmall,
                tc.tile_pool(name="projp", bufs=2) as projp,
                tc.tile_pool(name="ps_st", bufs=2, space="PSUM") as ps_st,
                tc.tile_pool(name="ps_y", bufs=1, space="PSUM") as ps_y,
                tc.tile_pool(name="ps_share", bufs=2, space="PSUM") as ps_share,
            ):
                # quarter-length x buffers, double-buffered across stages
                x_q = [
                    qkvp.tile([128, KT, T // 4], bf16, name=f"x_q{i}") for i in range(2)
                ]
                wqk_sb = qkvp.tile([128, KT, 2 * HL * D], bf16)
                wv_sb = qkvp.tile([128, KT, HL * D], bf16)
                for jt in range(4):
                    nc.sync.dma_start(
                        wqk_sb[:, :, jt * 128 : (jt + 1) * 128],
                        wqk[:, :, jt * 128 : (jt + 1) * 128],
                    )
                nc.sync.dma_start(wv_sb, wv[:, :, :])

                def qkv_stage(tb):
                    t0 = tb * 512
                    x_sb = x_q[tb % 2]
                    for kt in range(KT):
                        nc.sync.dma_start(x_sb[:, kt, :], x_in[:, kt, t0 : t0 + 512])
                    for jt in range(4):
                        qk_ps = ps_share.tile([128, 512], f32, tag="share", name="qk_ps")
                        for kt in range(KT):
                            nc.tensor.matmul(
                                qk_ps,
                                wqk_sb[:, kt, jt * 128 : (jt + 1) * 128],
                                x_sb[:, kt, :],
                                start=(kt == 0),
                                stop=(kt == KT - 1),
                            )
                        nc.vector.tensor_copy(qk_t[jt][tb], qk_ps)
                    for tt2 in range(4):
                        tt = tb * 4 + tt2
                        v_ps = ps_share.tile([128, HL * D], f32, tag="share", name="v_ps")
                        for kt in range(KT):
                            nc.tensor.matmul(
                                v_ps,
                                x_sb[:, kt, tt2 * 128 : (tt2 + 1) * 128],
                                wv_sb[:, kt, :],
                                start=(kt == 0),
                                stop=(kt == KT - 1),
                            )
                        nc.vector.tensor_copy(
                            v_t[tt][:, :, 0:D],
                            v_ps.rearrange("p (h d) -> p h d", h=HL),
                        )

                def proj_half(blk, half):
                    # c_proj for 2 token tiles of block blk; fills PE gaps
                    for tt in range(4 * blk + 2 * half, 4 * blk + 2 * half + 2):
                        o_sb = projp.tile([128, C], bf16, name="o_sb")
                        off = (tt % 4) * 128
                        for cb in range(2):
                            o_ps = ps_share.tile(
                                [128, 512], f32, tag="share", name="o_ps"
                            )
                            for pr in range(2):
                                nc.tensor.matmul(
                                    o_ps,
                                    y2_t[blk][:, pr, off : off + 128],
                                    wp_sb[:, pr, cb * 512 : (cb + 1) * 512],
                                    start=(pr == 0),
                                    stop=(pr == 1),
                                )
                            nc.vector.tensor_copy(
                                o_sb[:, cb * 512 : (cb + 1) * 512], o_ps
                            )
                        nc.sync.dma_start(out[tt * 128 : (tt + 1) * 128, :], o_sb)

                def attention_row(jq, hp):
                    njt = 4 * (jq + 1)
                    h0, h1 = 2 * hp, 2 * hp + 1
                    kslot, qslot = 2 + hp, hp
                    y2ps = ps_y.tile([D + 1, 2, 512], f32, name="y2ps")

                    def s_pair(j):
                        # both heads concurrently: 64-row tiles (0,0)+(64,0)
                        w = max(0, (j - 4 * jq) * 128)
                        st = ps_st.tile([128, 2, 512], f32, name="st")
                        diag = j >= 4 * jq
                        for s, pbase in ((0, 0), (1, 64)):
                            nc.tensor.matmul(
                                st[:, s, w:],
                                qk_t[kslot][j // 4][
                                    pbase : pbase + D,
                                    (j % 4) * 128 : (j % 4 + 1) * 128,
                                ],
                                qk_t[qslot][jq][pbase : pbase + D, w:],
                                start=True,
                                stop=not diag,
                                tile_position=(pbase, 0),
                            )
                        return st

                    def mask_pair(j, st):
                        if j >= 4 * jq:
                            w = (j - 4 * jq) * 128
                            for s in range(2):
                                nc.tensor.matmul(
                                    st[:, s, w : w + 128],
                                    negI,
                                    triM,
                                    start=False,
                                    stop=True,
                                )

                    def exp_block(j, st):
                        w = max(0, (j - 4 * jq) * 128)
                        est = attp.tile([128, 2, 512], bf16, tag="est", name="est")
                        nc.scalar.activation(
                            est[:, :, w:], st[:, :, w:], EXP, scale=SCALE
                        )
                        return est

                    def pv_block(j, est):
                        w = max(0, (j - 4 * jq) * 128)
                        for s, h in ((0, h0), (1, h1)):
                            nc.tensor.matmul(
                                y2ps[:, s, w:],
                                v_t[j][:, h, :],
                                est[:, s, w:],
                                start=(j == 0),
                                stop=(j == njt - 1),
                            )

                    # batches of 2 key blocks: S-pairs stay in 64-row tiling
                    # mode back-to-back, masks+PVs in 128 mode — one mode
                    # round-trip per batch instead of per block.  PVs lag one
                    # batch so the PE never waits on the ACT exp.
                    ests = [None] * njt
                    for jb in range(0, njt, 2):
                        stA = s_pair(jb)
                        stB = s_pair(jb + 1)
                        mask_pair(jb, stA)
                        mask_pair(jb + 1, stB)
                        ests[jb] = exp_block(jb, stA)
                        ests[jb + 1] = exp_block(jb + 1, stB)
                        if jb >= 2:
                            pv_block(jb - 2, ests[jb - 2])
                            pv_block(jb - 1, ests[jb - 1])
                    pv_block(njt - 2, ests[njt - 2])
                    pv_block(njt - 1, ests[njt - 1])

                    # epilogue: normalize by the denominator in row D.
                    # denom rows -> row D of a zeroed f32r tile -> K=128 PE
                    # broadcast (full-array mode) -> DVE reciprocal -> multiply.
                    rsb = rsb_slots[(2 * jq + hp) % 2]
                    nc.vector.tensor_copy(
                        rsb[D : D + 1, :, :], y2ps[D : D + 1, :, :]
                    )
                    rb_sb = attsmall.tile([D, 2, 512], f32, tag="rbs")
                    for s in range(2):
                        rb_ps = ps_share.tile(
                            [D, 512], f32, tag="share", name=f"rb{s}"
                        )
                        nc.tensor.matmul(
                            rb_ps,
                            ones_sb[:, :],
                            rsb[:, s, :],
                            start=True,
                            stop=True,
                        )
                        nc.vector.reciprocal_approx_fast(rb_sb[:, s, :], rb_ps)
                    nc.vector.tensor_mul(
                        y2_t[jq][0:D, hp, :], y2ps[0:D, 0, :], rb_sb[:, 0, :]
                    )
                    y_lo = attsmall.tile([D, 512], bf16, tag="ylo")
                    nc.vector.tensor_mul(y_lo, y2ps[0:D, 1, :], rb_sb[:, 1, :])
                    nc.gpsimd.dma_start(y2_t[jq][D:128, hp, :], y_lo)

                # interleave: attention for row jq is emitted right after its
                # qkv stage so the ACT exp stream starts ~15us in instead of
                # waiting for the whole qkv phase.
                for tb in range(4):
                    qkv_stage(tb)
                    for hp in range(2):
                        attention_row(tb, hp)
                        if tb > 0:
                            proj_half(tb - 1, hp)
                for hp in range(2):
                    proj_half(3, hp)

    nc.compile()
    return nc


def _get_nc():
    if "nc" not in _CACHE:
        _CACHE["nc"] = _build()
    return _CACHE["nc"]


def make_in_maps(x, w_attn, w_proj):
    import ml_dtypes

    bf16 = ml_dtypes.bfloat16
    x = np.asarray(x, np.float32)
    w_attn = np.asarray(w_attn, np.float32)
    w_proj = np.asarray(w_proj, np.float32)

    negI = np.zeros((128, 128), np.float32)
    np.fill_diagonal(negI, NEG)
    triM = np.tril(np.ones((128, 128), np.float32), -1)
    consts = np.concatenate([negI, triM], axis=1).astype(bf16)
    ones64 = np.ones((128, 64), np.float32)

    in_maps = []
    for c in range(N_CORES):
        b, hg = c // 4, c % 4
        hs = hg * HL * D  # 256 * hg
        xt = np.ascontiguousarray(x[b].T)  # [C, T]
        x_t = xt.reshape(KT, 128, T).transpose(1, 0, 2)
        wq = w_attn[hs : hs + HL * D, :]
        wk = w_attn[C + hs : C + hs + HL * D, :]
        wqkt = np.concatenate([wq, wk], 0).T  # [C, 512]
        wqk_t = wqkt.reshape(KT, 128, 2 * HL * D).transpose(1, 0, 2)
        wvt = w_attn[2 * C + hs : 2 * C + hs + HL * D, :].T  # [C, 256]
        wv_t = wvt.reshape(KT, 128, HL * D).transpose(1, 0, 2)
        # head-pair stacked rows: [128, HL//2, C]; partition p of pair pr is
        # local feature pr*128 + p (head 2*pr dims then head 2*pr+1 dims)
        wp_t = (
            w_proj[:, hs : hs + HL * D].T.reshape(HL // 2, 128, C).transpose(1, 0, 2)
        )
        in_maps.append(
            {
                "x_in": np.ascontiguousarray(x_t).astype(bf16),
                "wqk": np.ascontiguousarray(wqk_t).astype(bf16),
                "wv": np.ascontiguousarray(wv_t).astype(bf16),
                "wp": np.ascontiguousarray(wp_t).astype(bf16),
                "consts": consts,
                "ones64": ones64,
            }
        )
    return in_maps


def run(in_maps, **kwargs):
    nc = _get_nc()
    return run_bass_kernel_spmd(nc, in_maps, core_ids=list(range(N_CORES)), **kwargs)


def combine(results):
    out = np.zeros((B, T, C), np.float64)
    for c in range(N_CORES):
        out[c // 4] += results[c]["out"].astype(np.float64)
    return out.astype(np.float32)


def kernel(x, w_attn, w_proj):
    res = run(make_in_maps(x, w_attn, w_proj))
    return combine(res.results)


# revision 12
# speedup vs baseline: 1.1266x; 1.1266x over previous
"""Causal self-attention on 8 Trainium2 NeuronCores.

Sharding (batch + head parallel): core c handles batch b = c // 4 and the
4 heads [hg*4, hg*4+4) where hg = c % 4.  Each core computes q/k/v from
column-sliced c_attn weights, full causal attention for its heads, and a
partial c_proj output from the matching row slice of w_proj; the host sums
the 4 partials per batch.

All matmuls run in bf16 (fp32 PSUM accumulate).  The two heads of a pair
are computed CONCURRENTLY in the PE array for the S = K^T Q matmuls via
row tiling (K=64 contraction each, tile_position rows 0-63 / 64-127); S
batches of two key blocks keep the PE in 64-row tiling mode back-to-back
so the mode-switch drain is paid once per batch.  The causal mask is
applied inside PSUM with a -30000*tril matmul accumulate, so the
S -> exp -> PV chain only touches PE and ACT.

Emission WEAVES attention chunks with independent PE filler work (the
next stage's qkv matmul groups and c_proj halves) so the ACT exp stream
- the second-busiest engine - runs from ~15us in, and the PE always has
independent work queued behind ACT-dependent chunks (no head-of-line
stalls, HAM clock-gate stays at 2.4 GHz).
"""

import sys

if "/opt/trn_rl_repo" not in sys.path:
    sys.path.insert(0, "/opt/trn_rl_repo")

import numpy as np

import concourse.mybir as mybir
from concourse import bacc
from concourse.bass_utils import run_bass_kernel_spmd
from concourse.tile import TileContext

B, T, C = 2, 2048, 1024
H, D = 16, 64
HL = 4  # heads per core
N_CORES = 8
KT = C // 128  # contraction tiles over the embedding dim
SCALE = 1.0 / 8.0  # 1/sqrt(D)
NEG = -30000.0

_CACHE = {}


def _build():
    f32 = mybir.dt.float32
    f32r = mybir.dt.float32r
    bf16 = mybir.dt.bfloat16
    nc = bacc.Bacc("TRN2", target_bir_lowering=False, debug=False, num_devices=N_CORES)

    x_in = nc.dram_tensor("x_in", [128, KT, T], bf16, kind="ExternalInput")
    wqk = nc.dram_tensor("wqk", [128, KT, 2 * HL * D], bf16, kind="ExternalInput")
    wv = nc.dram_tensor("wv", [128, KT, HL * D], bf16, kind="ExternalInput")
    wp = nc.dram_tensor("wp", [128, HL // 2, C], bf16, kind="ExternalInput")
    # consts: cols 0:128 = diag(-30000); cols 128:256 = tril(ones, -1)
    consts = nc.dram_tensor("consts", [128, 256], bf16, kind="ExternalInput")
    ones64 = nc.dram_tensor("ones64", [128, 64], f32r, kind="ExternalInput")
    out = nc.dram_tensor("out", [T, C], bf16, kind="ExternalOutput")

    EXP = mybir.ActivationFunctionType.Exp

    with TileContext(nc) as tc:
        with tc.tile_pool(name="persist", bufs=1) as persist:
            # q/k feature-major [d, t]: slot 0/1 = q head-pairs 0/1, 2/3 = k;
            # partitions 0-63 = even head dims, 64-127 = odd head dims.
            qk_t = [
                [persist.tile([128, 512], bf16, name=f"qk{s}_{tb}") for tb in range(4)]
                for s in range(4)
            ]
            # v token-major per 128-token tile; col D holds ones (denominator)
            v_t = [
                persist.tile([128, HL, D + 1], bf16, name=f"v{tt}") for tt in range(16)
            ]
            # head-pair stacked normalized y per 512-token block
            y2_t = [
                persist.tile([128, HL // 2, 512], bf16, name=f"y2{b_}")
                for b_ in range(4)
            ]
            wp_sb = persist.tile([128, HL // 2, C], bf16)
            nc.sync.dma_start(wp_sb, wp[:, :, :])
            consts_sb = persist.tile([128, 256], bf16)
            nc.sync.dma_start(consts_sb, consts[:, :])
            negI = consts_sb[:, 0:128]
            triM = consts_sb[:, 128:256]
            ones_sb = persist.tile([128, 64], f32r)
            nc.sync.dma_start(ones_sb, ones64[:, :])
            # denominator staging rows: all partitions zero except row D, so
            # the broadcast matmul can contract K=128 (full-array mode, no
            # tiling-mode switch/drain on the PE)
            rsb_slots = [
                persist.tile([128, 2, 512], f32r, name=f"rsb{i}") for i in range(2)
            ]
            for r_ in rsb_slots:
                nc.gpsimd.memset(r_.bitcast(mybir.dt.uint32), 0)

            for tt in range(16):
                nc.gpsimd.memset(v_t[tt][:, :, D : D + 1], 1.0)

            with (
                tc.tile_pool(name="qkvp", bufs=1) as qkvp,
                tc.tile_pool(name="attp", bufs=6) as attp,
                tc.tile_pool(name="attsmall", bufs=2) as attsmall,
                tc.tile_pool(name="projp", bufs=2) as projp,
                tc.tile_pool(name="ps_st", bufs=2, space="PSUM") as ps_st,
                tc.tile_pool(name="ps_y", bufs=1, space="PSUM") as ps_y,
                tc.tile_pool(name="ps_share", bufs=2, space="PSUM") as ps_share,
            ):
                # quarter-length x buffers, double-buffered across stages
                x_q = [
                    qkvp.tile([128, KT, T // 4], bf16, name=f"x_q{i}") for i in range(2)
                ]
                wqk_sb = qkvp.tile([128, KT, 2 * HL * D], bf16)
                wv_sb = qkvp.tile([128, KT, HL * D], bf16)
                for jt in range(4):
                    nc.sync.dma_start(
                        wqk_sb[:, :, jt * 128 : (jt + 1) * 128],
                        wqk[:, :, jt * 128 : (jt + 1) * 128],
                    )
                nc.sync.dma_start(wv_sb, wv[:, :, :])

                def stage_dma(tb):
                    t0 = tb * 512
                    x_sb = x_q[tb % 2]
                    for kt in range(KT):
                        nc.sync.dma_start(x_sb[:, kt, :], x_in[:, kt, t0 : t0 + 512])

                def qk_group(tb, jt):
                    def go():
                        x_sb = x_q[tb % 2]
                        qk_ps = ps_share.tile(
                            [128, 512], f32, tag="share", name="qk_ps"
                        )
                        for kt in range(KT):
                            nc.tensor.matmul(
                                qk_ps,
                                wqk_sb[:, kt, jt * 128 : (jt + 1) * 128],
                                x_sb[:, kt, :],
                                start=(kt == 0),
                                stop=(kt == KT - 1),
                            )
                        nc.vector.tensor_copy(qk_t[jt][tb], qk_ps)

                    return go

                def v_group(tb, tt2):
                    def go():
                        x_sb = x_q[tb % 2]
                        tt = tb * 4 + tt2
                        v_ps = ps_share.tile(
                            [128, HL * D], f32, tag="share", name="v_ps"
                        )
                        for kt in range(KT):
                            nc.tensor.matmul(
                                v_ps,
                                x_sb[:, kt, tt2 * 128 : (tt2 + 1) * 128],
                                wv_sb[:, kt, :],
                                start=(kt == 0),
                                stop=(kt == KT - 1),
                            )
                        nc.vector.tensor_copy(
                            v_t[tt][:, :, 0:D],
                            v_ps.rearrange("p (h d) -> p h d", h=HL),
                        )

                    return go

                def stage_groups(tb):
                    # zipped so q and k/v land evenly through the row
                    return [qk_group(tb, 0), qk_group(tb, 1), v_group(tb, 0),
                            qk_group(tb, 2), v_group(tb, 1), qk_group(tb, 3),
                            v_group(tb, 2), v_group(tb, 3)]

                def proj_tile(blk, tt):
                    def go():
                        o_sb = projp.tile([128, C], bf16, name="o_sb")
                        off = (tt % 4) * 128
                        for cb in range(2):
                            o_ps = ps_share.tile(
                                [128, 512], f32, tag="share", name="o_ps"
                            )
                            for pr in range(2):
                                nc.tensor.matmul(
                                    o_ps,
                                    y2_t[blk][:, pr, off : off + 128],
                                    wp_sb[:, pr, cb * 512 : (cb + 1) * 512],
                                    start=(pr == 0),
                                    stop=(pr == 1),
                                )
                            nc.vector.tensor_copy(
                                o_sb[:, cb * 512 : (cb + 1) * 512], o_ps
                            )
                        nc.sync.dma_start(out[tt * 128 : (tt + 1) * 128, :], o_sb)

                    return go

                def attention_chunks(jq):
                    njt = 4 * (jq + 1)
                    chunks = []

                    def build_hp(hp):
                        # separate scope per hp: closures run later (in the
                        # weave), so they must bind this hp's values now
                        h0, h1 = 2 * hp, 2 * hp + 1
                        kslot, qslot = 2 + hp, hp
                        state = {"ests": [None] * njt}

                        def s_pair(j, state):
                            w = max(0, (j - 4 * jq) * 128)
                            st = ps_st.tile([128, 2, 512], f32, name="st")
                            diag = j >= 4 * jq
                            for s, pbase in ((0, 0), (1, 64)):
                                nc.tensor.matmul(
                                    st[:, s, w:],
                                    qk_t[kslot][j // 4][
                                        pbase : pbase + D,
                                        (j % 4) * 128 : (j % 4 + 1) * 128,
                                    ],
                                    qk_t[qslot][jq][pbase : pbase + D, w:],
                                    start=True,
                                    stop=not diag,
                                    tile_position=(pbase, 0),
                                )
                            return st

                        def mask_pair(j, st):
                            if j >= 4 * jq:
                                w = (j - 4 * jq) * 128
                                for s in range(2):
                                    nc.tensor.matmul(
                                        st[:, s, w : w + 128],
                                        negI,
                                        triM,
                                        start=False,
                                        stop=True,
                                    )

                        def exp_block(j, st):
                            w = max(0, (j - 4 * jq) * 128)
                            est = attp.tile(
                                [128, 2, 512], bf16, tag="est", name="est"
                            )
                            nc.scalar.activation(
                                est[:, :, w:], st[:, :, w:], EXP, scale=SCALE
                            )
                            return est

                        def pv_block(j, state):
                            w = max(0, (j - 4 * jq) * 128)
                            for s, h in ((0, h0), (1, h1)):
                                nc.tensor.matmul(
                                    state["y2ps"][:, s, w:],
                                    v_t[j][:, h, :],
                                    state["ests"][j][:, s, w:],
                                    start=(j == 0),
                                    stop=(j == njt - 1),
                                )

                        def batch(jb, state=state):
                            def go():
                                if jb == 0:
                                    state["y2ps"] = ps_y.tile(
                                        [D + 1, 2, 512], f32, name="y2ps"
                                    )
                                stA = s_pair(jb, state)
                                stB = s_pair(jb + 1, state)
                                mask_pair(jb, stA)
                                mask_pair(jb + 1, stB)
                                state["ests"][jb] = exp_block(jb, stA)
                                state["ests"][jb + 1] = exp_block(jb + 1, stB)
                                if jb >= 2:
                                    pv_block(jb - 2, state)
                                    pv_block(jb - 1, state)

                            return go

                        def tail(state=state, hp=hp):
                            def go():
                                pv_block(njt - 2, state)
                                pv_block(njt - 1, state)
                                # epi part 1: denominator rows -> f32r sbuf
                                rsb = rsb_slots[(2 * jq + hp) % 2]
                                nc.vector.tensor_copy(
                                    rsb[D : D + 1, :, :],
                                    state["y2ps"][D : D + 1, :, :],
                                )

                            return go

                        def epi2(state=state, hp=hp):
                            def go():
                                # K=128 broadcast (full-array mode) -> DVE
                                # reciprocal -> psum-by-sbuf normalize.
                                rsb = rsb_slots[(2 * jq + hp) % 2]
                                y2ps = state["y2ps"]
                                rb_sb = attsmall.tile([D, 2, 512], f32, tag="rbs")
                                for s in range(2):
                                    rb_ps = ps_share.tile(
                                        [D, 512], f32, tag="share", name=f"rb{s}"
                                    )
                                    nc.tensor.matmul(
                                        rb_ps,
                                        ones_sb[:, :],
                                        rsb[:, s, :],
                                        start=True,
                                        stop=True,
                                    )
                                    nc.vector.reciprocal_approx_fast(
                                        rb_sb[:, s, :], rb_ps
                                    )
                                nc.vector.tensor_mul(
                                    y2_t[jq][0:D, hp, :],
                                    y2ps[0:D, 0, :],
                                    rb_sb[:, 0, :],
                                )
                                y_lo = attsmall.tile([D, 512], bf16, tag="ylo")
                                nc.vector.tensor_mul(
                                    y_lo, y2ps[0:D, 1, :], rb_sb[:, 1, :]
                                )
                                nc.gpsimd.dma_start(
                                    y2_t[jq][D:128, hp, :], y_lo
                                )

                            return go

                        for jb in range(0, njt, 2):
                            chunks.append(batch(jb))
                        chunks.append(tail())
                        chunks.append(epi2())

                    for hp in range(2):
                        build_hp(hp)
                    return chunks

                def weave(chunks, fillers):
                    n, m = len(chunks), len(fillers)
                    fi = 0
                    for i, ch in enumerate(chunks):
                        ch()
                        want = (i + 1) * m // n
                        while fi < want:
                            fillers[fi]()
                            fi += 1

                # serial head: stage 0 qkv
                stage_dma(0)
                for g in stage_groups(0):
                    g()
                # each attention row jq=tb is woven with the next stage's qkv
                # groups and the previous block's c_proj as PE filler
                for tb in range(4):
                    fillers = []
                    if tb < 3:
                        stage_dma(tb + 1)
                        fillers += stage_groups(tb + 1)
                    if tb >= 1:
                        fillers += [proj_tile(tb - 1, 4 * (tb - 1) + i)
                                    for i in range(4)]
                    weave(attention_chunks(tb), fillers)
                for i in range(4):
                    proj_tile(3, 12 + i)()

    nc.compile()
    return nc


def _get_nc():
    if "nc" not in _CACHE:
        _CACHE["nc"] = _build()
    return _CACHE["nc"]


def make_in_maps(x, w_attn, w_proj):
    import ml_dtypes

    bf16 = ml_dtypes.bfloat16
    x = np.asarray(x, np.float32)
    w_attn = np.asarray(w_attn, np.float32)
    w_proj = np.asarray(w_proj, np.float32)

    negI = np.zeros((128, 128), np.float32)
    np.fill_diagonal(negI, NEG)
    triM = np.tril(np.ones((128, 128), np.float32), -1)
    consts = np.concatenate([negI, triM], axis=1).astype(bf16)
    ones64 = np.ones((128, 64), np.float32)

    in_maps = []
    for c in range(N_CORES):
        b, hg = c // 4, c % 4
        hs = hg * HL * D  # 256 * hg
        xt = np.ascontiguousarray(x[b].T)  # [C, T]
        x_t = xt.reshape(KT, 128, T).transpose(1, 0, 2)
        wq = w_attn[hs : hs + HL * D, :]
        wk = w_attn[C + hs : C + hs + HL * D, :]
        wqkt = np.concatenate([wq, wk], 0).T  # [C, 512]
        wqk_t = wqkt.reshape(KT, 128, 2 * HL * D).transpose(1, 0, 2)
        wvt = w_attn[2 * C + hs : 2 * C + hs + HL * D, :].T  # [C, 256]
        wv_t = wvt.reshape(KT, 128, HL * D).transpose(1, 0, 2)
        # head-pair stacked rows: [128, HL//2, C]; partition p of pair pr is
        # local feature pr*128 + p (head 2*pr dims then head 2*pr+1 dims)
        wp_t = (
            w_proj[:, hs : hs + HL * D].T.reshape(HL // 2, 128, C).transpose(1, 0, 2)
        )
        in_maps.append(
            {
                "x_in": np.ascontiguousarray(x_t).astype(bf16),
                "wqk": np.ascontiguousarray(wqk_t).astype(bf16),
                "wv": np.ascontiguousarray(wv_t).astype(bf16),
                "wp": np.ascontiguousarray(wp_t).astype(bf16),
                "consts": consts,
                "ones64": ones64,
            }
        )
    return in_maps


def run(in_maps, **kwargs):
    nc = _get_nc()
    return run_bass_kernel_spmd(nc, in_maps, core_ids=list(range(N_CORES)), **kwargs)


def combine(results):
    out = np.zeros((B, T, C), np.float64)
    for c in range(N_CORES):
        out[c // 4] += results[c]["out"].astype(np.float64)
    return out.astype(np.float32)


def kernel(x, w_attn, w_proj):
    res = run(make_in_maps(x, w_attn, w_proj))
    return combine(res.results)


# revision 16
# speedup vs baseline: 1.1326x; 1.0052x over previous
"""Causal self-attention on 8 Trainium2 NeuronCores.

Sharding (batch + head parallel): core c handles batch b = c // 4 and the
4 heads [hg*4, hg*4+4) where hg = c % 4.  Each core computes q/k/v from
column-sliced c_attn weights, full causal attention for its heads, and a
partial c_proj output from the matching row slice of w_proj; the host sums
the 4 partials per batch.

All matmuls run in bf16 (fp32 PSUM accumulate).  The two heads of a pair
are computed CONCURRENTLY in the PE array for the S = K^T Q matmuls via
row tiling (K=64 contraction each, tile_position rows 0-63 / 64-127); S
batches of two key blocks keep the PE in 64-row tiling mode back-to-back
so the mode-switch drain is paid once per batch.  The causal mask is
applied inside PSUM with a -30000*tril matmul accumulate, so the
S -> exp -> PV chain only touches PE and ACT.

Emission WEAVES attention chunks with independent PE filler work (the
next stage's qkv matmul groups and c_proj halves) so the ACT exp stream
- the second-busiest engine - runs from ~15us in, and the PE always has
independent work queued behind ACT-dependent chunks (no head-of-line
stalls, HAM clock-gate stays at 2.4 GHz).
"""

import sys

if "/opt/trn_rl_repo" not in sys.path:
    sys.path.insert(0, "/opt/trn_rl_repo")

import numpy as np

import concourse.mybir as mybir
from concourse import bacc
from concourse.bass_utils import run_bass_kernel_spmd
from concourse.tile import TileContext

B, T, C = 2, 2048, 1024
H, D = 16, 64
HL = 4  # heads per core
N_CORES = 8
KT = C // 128  # contraction tiles over the embedding dim
SCALE = 1.0 / 8.0  # 1/sqrt(D)
NEG = -30000.0

_CACHE = {}


def _build():
    f32 = mybir.dt.float32
    f32r = mybir.dt.float32r
    bf16 = mybir.dt.bfloat16
    nc = bacc.Bacc("TRN2", target_bir_lowering=False, debug=False, num_devices=N_CORES)

    x_in = nc.dram_tensor("x_in", [128, KT, T], bf16, kind="ExternalInput")
    wqk = nc.dram_tensor("wqk", [128, KT, 2 * HL * D], bf16, kind="ExternalInput")
    wv = nc.dram_tensor("wv", [128, KT, HL * D], bf16, kind="ExternalInput")
    wp = nc.dram_tensor("wp", [128, HL // 2, C], bf16, kind="ExternalInput")
    # consts: cols 0:128 = diag(-30000); cols 128:256 = tril(ones, -1)
    consts = nc.dram_tensor("consts", [128, 256], bf16, kind="ExternalInput")
    ones64 = nc.dram_tensor("ones64", [128, 64], f32r, kind="ExternalInput")
    out = nc.dram_tensor("out", [T, C], bf16, kind="ExternalOutput")

    EXP = mybir.ActivationFunctionType.Exp

    with TileContext(nc) as tc:
        with tc.tile_pool(name="persist", bufs=1) as persist:
            # q/k feature-major [d, t]: slot 0/1 = q head-pairs 0/1, 2/3 = k;
            # partitions 0-63 = even head dims, 64-127 = odd head dims.
            qk_t = [
                [persist.tile([128, 512], bf16, name=f"qk{s}_{tb}") for tb in range(4)]
                for s in range(4)
            ]
            # v token-major per 128-token tile; col D holds ones (denominator)
            v_t = [
                persist.tile([128, HL, D + 1], bf16, name=f"v{tt}") for tt in range(16)
            ]
            # head-pair stacked normalized y per 512-token block
            y2_t = [
                persist.tile([128, HL // 2, 512], bf16, name=f"y2{b_}")
                for b_ in range(4)
            ]
            wp_sb = persist.tile([128, HL // 2, C], bf16)
            consts_sb = persist.tile([128, 256], bf16)
            negI = consts_sb[:, 0:128]
            triM = consts_sb[:, 128:256]
            ones_sb = persist.tile([128, 64], f32r)
            # denominator staging rows: all partitions zero except row D, so
            # the broadcast matmul can contract K=128 (full-array mode, no
            # tiling-mode switch/drain on the PE)
            rsb_slots = [
                persist.tile([128, 2, 512], f32r, name=f"rsb{i}") for i in range(2)
            ]
            for r_ in rsb_slots:
                nc.gpsimd.memset(r_.bitcast(mybir.dt.uint32), 0)

            for tt in range(16):
                nc.gpsimd.memset(v_t[tt][:, :, D : D + 1], 1.0)

            with (
                tc.tile_pool(name="qkvp", bufs=1) as qkvp,
                tc.tile_pool(name="attp", bufs=6) as attp,
                tc.tile_pool(name="attsmall", bufs=2) as attsmall,
                tc.tile_pool(name="projp", bufs=2) as projp,
                tc.tile_pool(name="ps_st", bufs=2, space="PSUM") as ps_st,
                tc.tile_pool(name="ps_y", bufs=1, space="PSUM") as ps_y,
                tc.tile_pool(name="ps_share", bufs=2, space="PSUM") as ps_share,
            ):
                # quarter-length x buffers, double-buffered across stages
                x_q = [
                    qkvp.tile([128, KT, T // 4], bf16, name=f"x_q{i}") for i in range(2)
                ]
                wqk_sb = qkvp.tile([128, KT, 2 * HL * D], bf16)
                wv_sb = qkvp.tile([128, KT, HL * D], bf16)

                def stage_dma(tb):
                    t0 = tb * 512
                    x_sb = x_q[tb % 2]
                    for kt in range(KT):
                        nc.sync.dma_start(x_sb[:, kt, :], x_in[:, kt, t0 : t0 + 512])

                def qk_group(tb, jt):
                    def go():
                        x_sb = x_q[tb % 2]
                        qk_ps = ps_share.tile(
                            [128, 512], f32, tag="share", name="qk_ps"
                        )
                        for kt in range(KT):
                            nc.tensor.matmul(
                                qk_ps,
                                wqk_sb[:, kt, jt * 128 : (jt + 1) * 128],
                                x_sb[:, kt, :],
                                start=(kt == 0),
                                stop=(kt == KT - 1),
                            )
                        nc.vector.tensor_copy(qk_t[jt][tb], qk_ps)

                    return go

                def v_group(tb, tt2):
                    def go():
                        x_sb = x_q[tb % 2]
                        tt = tb * 4 + tt2
                        v_ps = ps_share.tile(
                            [128, HL * D], f32, tag="share", name="v_ps"
                        )
                        for kt in range(KT):
                            nc.tensor.matmul(
                                v_ps,
                                x_sb[:, kt, tt2 * 128 : (tt2 + 1) * 128],
                                wv_sb[:, kt, :],
                                start=(kt == 0),
                                stop=(kt == KT - 1),
                            )
                        nc.vector.tensor_copy(
                            v_t[tt][:, :, 0:D],
                            v_ps.rearrange("p (h d) -> p h d", h=HL),
                        )

                    return go

                def stage_groups(tb):
                    # zipped so q and k/v land evenly through the row
                    return [qk_group(tb, 0), qk_group(tb, 1), v_group(tb, 0),
                            qk_group(tb, 2), v_group(tb, 1), qk_group(tb, 3),
                            v_group(tb, 2), v_group(tb, 3)]

                def proj_tile(blk, tt):
                    def go():
                        o_sb = projp.tile([128, C], bf16, name="o_sb")
                        off = (tt % 4) * 128
                        for cb in range(2):
                            o_ps = ps_share.tile(
                                [128, 512], f32, tag="share", name="o_ps"
                            )
                            for pr in range(2):
                                nc.tensor.matmul(
                                    o_ps,
                                    y2_t[blk][:, pr, off : off + 128],
                                    wp_sb[:, pr, cb * 512 : (cb + 1) * 512],
                                    start=(pr == 0),
                                    stop=(pr == 1),
                                )
                            # the last block runs after the exp stream ends:
                            # evacuate on the then-idle ACT instead of DVE
                            if blk == 3:
                                nc.scalar.copy(
                                    o_sb[:, cb * 512 : (cb + 1) * 512], o_ps
                                )
                            else:
                                nc.vector.tensor_copy(
                                    o_sb[:, cb * 512 : (cb + 1) * 512], o_ps
                                )
                        nc.sync.dma_start(out[tt * 128 : (tt + 1) * 128, :], o_sb)

                    return go

                def attention_chunks(jq):
                    njt = 4 * (jq + 1)
                    chunks = []

                    def build_hp(hp):
                        # separate scope per hp: closures run later (in the
                        # weave), so they must bind this hp's values now
                        h0, h1 = 2 * hp, 2 * hp + 1
                        kslot, qslot = 2 + hp, hp
                        state = {"ests": [None] * njt}

                        def s_pair(j, state):
                            w = max(0, (j - 4 * jq) * 128)
                            st = ps_st.tile([128, 2, 512], f32, name="st")
                            diag = j >= 4 * jq
                            for s, pbase in ((0, 0), (1, 64)):
                                nc.tensor.matmul(
                                    st[:, s, w:],
                                    qk_t[kslot][j // 4][
                                        pbase : pbase + D,
                                        (j % 4) * 128 : (j % 4 + 1) * 128,
                                    ],
                                    qk_t[qslot][jq][pbase : pbase + D, w:],
                                    start=True,
                                    stop=not diag,
                                    tile_position=(pbase, 0),
                                )
                            return st

                        def mask_pair(j, st):
                            if j >= 4 * jq:
                                w = (j - 4 * jq) * 128
                                for s in range(2):
                                    nc.tensor.matmul(
                                        st[:, s, w : w + 128],
                                        negI,
                                        triM,
                                        start=False,
                                        stop=True,
                                    )

                        def exp_block(j, st):
                            w = max(0, (j - 4 * jq) * 128)
                            est = attp.tile(
                                [128, 2, 512], bf16, tag="est", name="est"
                            )
                            nc.scalar.activation(
                                est[:, :, w:], st[:, :, w:], EXP, scale=SCALE
                            )
                            return est

                        def pv_block(j, state):
                            w = max(0, (j - 4 * jq) * 128)
                            for s, h in ((0, h0), (1, h1)):
                                nc.tensor.matmul(
                                    state["y2ps"][:, s, w:],
                                    v_t[j][:, h, :],
                                    state["ests"][j][:, s, w:],
                                    start=(j == 0),
                                    stop=(j == njt - 1),
                                )

                        def batch(jb, state=state):
                            def go():
                                if jb == 0:
                                    state["y2ps"] = ps_y.tile(
                                        [D + 1, 2, 512], f32, name="y2ps"
                                    )
                                stA = s_pair(jb, state)
                                stB = s_pair(jb + 1, state)
                                mask_pair(jb, stA)
                                mask_pair(jb + 1, stB)
                                state["ests"][jb] = exp_block(jb, stA)
                                state["ests"][jb + 1] = exp_block(jb + 1, stB)
                                if jb >= 2:
                                    pv_block(jb - 2, state)
                                    pv_block(jb - 1, state)

                            return go

                        def tail(state=state, hp=hp):
                            def go():
                                pv_block(njt - 2, state)
                                pv_block(njt - 1, state)
                                # epi part 1: denominator rows -> f32r sbuf
                                rsb = rsb_slots[(2 * jq + hp) % 2]
                                nc.vector.tensor_copy(
                                    rsb[D : D + 1, :, :],
                                    state["y2ps"][D : D + 1, :, :],
                                )

                            return go

                        def epi2(state=state, hp=hp):
                            def go():
                                # K=128 broadcast (full-array mode) -> DVE
                                # reciprocal -> psum-by-sbuf normalize.
                                rsb = rsb_slots[(2 * jq + hp) % 2]
                                y2ps = state["y2ps"]
                                rb_sb = attsmall.tile([D, 2, 512], f32, tag="rbs")
                                for s in range(2):
                                    rb_ps = ps_share.tile(
                                        [D, 512], f32, tag="share", name=f"rb{s}"
                                    )
                                    nc.tensor.matmul(
                                        rb_ps,
                                        ones_sb[:, :],
                                        rsb[:, s, :],
                                        start=True,
                                        stop=True,
                                    )
                                    nc.vector.reciprocal_approx_fast(
                                        rb_sb[:, s, :], rb_ps
                                    )
                                nc.vector.tensor_mul(
                                    y2_t[jq][0:D, hp, :],
                                    y2ps[0:D, 0, :],
                                    rb_sb[:, 0, :],
                                )
                                y_lo = attsmall.tile([D, 512], bf16, tag="ylo")
                                nc.vector.tensor_mul(
                                    y_lo, y2ps[0:D, 1, :], rb_sb[:, 1, :]
                                )
                                nc.gpsimd.dma_start(
                                    y2_t[jq][D:128, hp, :], y_lo
                                )

                            return go

                        for jb in range(0, njt, 2):
                            chunks.append(batch(jb))
                        chunks.append(tail())
                        chunks.append(epi2())

                    for hp in range(2):
                        build_hp(hp)
                    return chunks

                def weave(chunks, fillers):
                    n, m = len(chunks), len(fillers)
                    fi = 0
                    for i, ch in enumerate(chunks):
                        ch()
                        want = (i + 1) * m // n
                        while fi < want:
                            fillers[fi]()
                            fi += 1

                # DMA order matters for the kernel head: the data the first
                # matmuls need (stage-0 x, wqk) goes first; wp/consts are
                # only needed tens of us in.
                stage_dma(0)
                for jt in range(4):
                    nc.sync.dma_start(
                        wqk_sb[:, :, jt * 128 : (jt + 1) * 128],
                        wqk[:, :, jt * 128 : (jt + 1) * 128],
                    )
                nc.sync.dma_start(wv_sb, wv[:, :, :])
                nc.sync.dma_start(consts_sb, consts[:, :])
                nc.sync.dma_start(ones_sb, ones64[:, :])
                nc.sync.dma_start(wp_sb, wp[:, :, :])

                # serial head: stage 0 qkv
                for g in stage_groups(0):
                    g()
                # each attention row jq=tb is woven with the next stage's qkv
                # groups and the previous block's c_proj as PE filler
                for tb in range(4):
                    fillers = []
                    if tb < 3:
                        stage_dma(tb + 1)
                        fillers += stage_groups(tb + 1)
                    if tb >= 1:
                        fillers += [proj_tile(tb - 1, 4 * (tb - 1) + i)
                                    for i in range(4)]
                    weave(attention_chunks(tb), fillers)
                for i in range(4):
                    proj_tile(3, 12 + i)()

    nc.compile()
    return nc


def _get_nc():
    if "nc" not in _CACHE:
        _CACHE["nc"] = _build()
    return _CACHE["nc"]


def make_in_maps(x, w_attn, w_proj):
    import ml_dtypes

    bf16 = ml_dtypes.bfloat16
    x = np.asarray(x, np.float32)
    w_attn = np.asarray(w_attn, np.float32)
    w_proj = np.asarray(w_proj, np.float32)

    negI = np.zeros((128, 128), np.float32)
    np.fill_diagonal(negI, NEG)
    triM = np.tril(np.ones((128, 128), np.float32), -1)
    consts = np.concatenate([negI, triM], axis=1).astype(bf16)
    ones64 = np.ones((128, 64), np.float32)

    in_maps = []
    for c in range(N_CORES):
        b, hg = c // 4, c % 4
        hs = hg * HL * D  # 256 * hg
        xt = np.ascontiguousarray(x[b].T)  # [C, T]
        x_t = xt.reshape(KT, 128, T).transpose(1, 0, 2)
        wq = w_attn[hs : hs + HL * D, :]
        wk = w_attn[C + hs : C + hs + HL * D, :]
        wqkt = np.concatenate([wq, wk], 0).T  # [C, 512]
        wqk_t = wqkt.reshape(KT, 128, 2 * HL * D).transpose(1, 0, 2)
        wvt = w_attn[2 * C + hs : 2 * C + hs + HL * D, :].T  # [C, 256]
        wv_t = wvt.reshape(KT, 128, HL * D).transpose(1, 0, 2)
        # head-pair stacked rows: [128, HL//2, C]; partition p of pair pr is
        # local feature pr*128 + p (head 2*pr dims then head 2*pr+1 dims)
        wp_t = (
            w_proj[:, hs : hs + HL * D].T.reshape(HL // 2, 128, C).transpose(1, 0, 2)
        )
        in_maps.append(
            {
                "x_in": np.ascontiguousarray(x_t).astype(bf16),
                "wqk": np.ascontiguousarray(wqk_t).astype(bf16),
                "wv": np.ascontiguousarray(wv_t).astype(bf16),
                "wp": np.ascontiguousarray(wp_t).astype(bf16),
                "consts": consts,
                "ones64": ones64,
            }
        )
    return in_maps


def run(in_maps, **kwargs):
    nc = _get_nc()
    return run_bass_kernel_spmd(nc, in_maps, core_ids=list(range(N_CORES)), **kwargs)


def combine(results):
    out = np.zeros((B, T, C), np.float64)
    for c in range(N_CORES):
        out[c // 4] += results[c]["out"].astype(np.float64)
    return out.astype(np.float32)


def kernel(x, w_attn, w_proj):
    res = run(make_in_maps(x, w_attn, w_proj))
    return combine(res.results)


# revision 17
# speedup vs baseline: 1.1518x; 1.0170x over previous
"""Causal self-attention on 8 Trainium2 NeuronCores.

Sharding (batch + head parallel): core c handles batch b = c // 4 and the
4 heads [hg*4, hg*4+4) where hg = c % 4.  Each core computes q/k/v from
column-sliced c_attn weights, full causal attention for its heads, and a
partial c_proj output from the matching row slice of w_proj; the host sums
the 4 partials per batch.

All matmuls run in bf16 (fp32 PSUM accumulate).  The two heads of a pair
are computed CONCURRENTLY in the PE array for the S = K^T Q matmuls via
row tiling (K=64 contraction each, tile_position rows 0-63 / 64-127); S
batches of two key blocks keep the PE in 64-row tiling mode back-to-back
so the mode-switch drain is paid once per batch.  The causal mask is
applied inside PSUM with a -30000*tril matmul accumulate, so the
S -> exp -> PV chain only touches PE and ACT.

Emission WEAVES attention chunks with independent PE filler work (the
next stage's qkv matmul groups and c_proj halves) so the ACT exp stream
- the second-busiest engine - runs from ~15us in, and the PE always has
independent work queued behind ACT-dependent chunks (no head-of-line
stalls, HAM clock-gate stays at 2.4 GHz).
"""

import sys

if "/opt/trn_rl_repo" not in sys.path:
    sys.path.insert(0, "/opt/trn_rl_repo")

import numpy as np

import concourse.mybir as mybir
from concourse import bacc
from concourse.bass_utils import run_bass_kernel_spmd
from concourse.tile import TileContext

B, T, C = 2, 2048, 1024
H, D = 16, 64
HL = 4  # heads per core
N_CORES = 8
KT = C // 128  # contraction tiles over the embedding dim
SCALE = 1.0 / 8.0  # 1/sqrt(D)
NEG = -30000.0

_CACHE = {}


def _build():
    f32 = mybir.dt.float32
    f32r = mybir.dt.float32r
    bf16 = mybir.dt.bfloat16
    nc = bacc.Bacc("TRN2", target_bir_lowering=False, debug=False, num_devices=N_CORES)

    x_in = nc.dram_tensor("x_in", [128, 4, KT, T // 4], bf16, kind="ExternalInput")
    wqk = nc.dram_tensor("wqk", [128, KT, 2 * HL * D], bf16, kind="ExternalInput")
    wv = nc.dram_tensor("wv", [128, KT, HL * D], bf16, kind="ExternalInput")
    wp = nc.dram_tensor("wp", [128, HL // 2, C], bf16, kind="ExternalInput")
    # consts: cols 0:128 = diag(-30000); cols 128:256 = tril(ones, -1)
    consts = nc.dram_tensor("consts", [128, 256], bf16, kind="ExternalInput")
    ones64 = nc.dram_tensor("ones64", [128, 64], f32r, kind="ExternalInput")
    out = nc.dram_tensor("out", [T, C], bf16, kind="ExternalOutput")

    EXP = mybir.ActivationFunctionType.Exp

    with TileContext(nc) as tc:
        with tc.tile_pool(name="persist", bufs=1) as persist:
            # q/k feature-major [d, t]: slot 0/1 = q head-pairs 0/1, 2/3 = k;
            # partitions 0-63 = even head dims, 64-127 = odd head dims.
            qk_t = [
                [persist.tile([128, 512], bf16, name=f"qk{s}_{tb}") for tb in range(4)]
                for s in range(4)
            ]
            # v token-major per 128-token tile; col D holds ones (denominator)
            v_t = [
                persist.tile([128, HL, D + 1], bf16, name=f"v{tt}") for tt in range(16)
            ]
            # head-pair stacked normalized y per 512-token block
            y2_t = [
                persist.tile([128, HL // 2, 512], bf16, name=f"y2{b_}")
                for b_ in range(4)
            ]
            wp_sb = persist.tile([128, HL // 2, C], bf16)
            consts_sb = persist.tile([128, 256], bf16)
            negI = consts_sb[:, 0:128]
            triM = consts_sb[:, 128:256]
            ones_sb = persist.tile([128, 64], f32r)
            # denominator staging rows: all partitions zero except row D, so
            # the broadcast matmul can contract K=128 (full-array mode, no
            # tiling-mode switch/drain on the PE)
            rsb_slots = [
                persist.tile([128, 2, 512], f32r, name=f"rsb{i}") for i in range(2)
            ]
            for r_ in rsb_slots:
                nc.gpsimd.memset(r_.bitcast(mybir.dt.uint32), 0)

            for tt in range(16):
                nc.gpsimd.memset(v_t[tt][:, :, D : D + 1], 1.0)

            with (
                tc.tile_pool(name="qkvp", bufs=1) as qkvp,
                tc.tile_pool(name="attp", bufs=6) as attp,
                tc.tile_pool(name="attsmall", bufs=2) as attsmall,
                tc.tile_pool(name="projp", bufs=2) as projp,
                tc.tile_pool(name="ps_st", bufs=2, space="PSUM") as ps_st,
                tc.tile_pool(name="ps_y", bufs=1, space="PSUM") as ps_y,
                tc.tile_pool(name="ps_share", bufs=2, space="PSUM") as ps_share,
            ):
                # quarter-length x buffers, double-buffered across stages
                x_q = [
                    qkvp.tile([128, KT, T // 4], bf16, name=f"x_q{i}") for i in range(2)
                ]
                wqk_sb = qkvp.tile([128, KT, 2 * HL * D], bf16)
                wv_sb = qkvp.tile([128, KT, HL * D], bf16)

                def stage_dma(tb):
                    # stage-major x layout: 8KB contiguous per partition ->
                    # few large DMA descriptors instead of ~1K small ones
                    nc.sync.dma_start(x_q[tb % 2], x_in[:, tb])

                def qk_group(tb, jt):
                    def go():
                        x_sb = x_q[tb % 2]
                        qk_ps = ps_share.tile(
                            [128, 512], f32, tag="share", name="qk_ps"
                        )
                        for kt in range(KT):
                            nc.tensor.matmul(
                                qk_ps,
                                wqk_sb[:, kt, jt * 128 : (jt + 1) * 128],
                                x_sb[:, kt, :],
                                start=(kt == 0),
                                stop=(kt == KT - 1),
                            )
                        nc.vector.tensor_copy(qk_t[jt][tb], qk_ps)

                    return go

                def v_group(tb, tt2):
                    def go():
                        x_sb = x_q[tb % 2]
                        tt = tb * 4 + tt2
                        v_ps = ps_share.tile(
                            [128, HL * D], f32, tag="share", name="v_ps"
                        )
                        for kt in range(KT):
                            nc.tensor.matmul(
                                v_ps,
                                x_sb[:, kt, tt2 * 128 : (tt2 + 1) * 128],
                                wv_sb[:, kt, :],
                                start=(kt == 0),
                                stop=(kt == KT - 1),
                            )
                        nc.vector.tensor_copy(
                            v_t[tt][:, :, 0:D],
                            v_ps.rearrange("p (h d) -> p h d", h=HL),
                        )

                    return go

                def stage_groups(tb):
                    # zipped so q and k/v land evenly through the row
                    return [qk_group(tb, 0), qk_group(tb, 1), v_group(tb, 0),
                            qk_group(tb, 2), v_group(tb, 1), qk_group(tb, 3),
                            v_group(tb, 2), v_group(tb, 3)]

                def proj_tile(blk, tt):
                    def go():
                        o_sb = projp.tile([128, C], bf16, name="o_sb")
                        off = (tt % 4) * 128
                        for cb in range(2):
                            o_ps = ps_share.tile(
                                [128, 512], f32, tag="share", name="o_ps"
                            )
                            for pr in range(2):
                                nc.tensor.matmul(
                                    o_ps,
                                    y2_t[blk][:, pr, off : off + 128],
                                    wp_sb[:, pr, cb * 512 : (cb + 1) * 512],
                                    start=(pr == 0),
                                    stop=(pr == 1),
                                )
                            # the last block runs after the exp stream ends:
                            # evacuate on the then-idle ACT instead of DVE
                            if blk == 3:
                                nc.scalar.copy(
                                    o_sb[:, cb * 512 : (cb + 1) * 512], o_ps
                                )
                            else:
                                nc.vector.tensor_copy(
                                    o_sb[:, cb * 512 : (cb + 1) * 512], o_ps
                                )
                        nc.sync.dma_start(out[tt * 128 : (tt + 1) * 128, :], o_sb)

                    return go

                def attention_chunks(jq):
                    njt = 4 * (jq + 1)
                    chunks = []

                    def build_hp(hp):
                        # separate scope per hp: closures run later (in the
                        # weave), so they must bind this hp's values now
                        h0, h1 = 2 * hp, 2 * hp + 1
                        kslot, qslot = 2 + hp, hp
                        state = {"ests": [None] * njt}

                        def s_pair(j, state):
                            w = max(0, (j - 4 * jq) * 128)
                            st = ps_st.tile([128, 2, 512], f32, name="st")
                            diag = j >= 4 * jq
                            for s, pbase in ((0, 0), (1, 64)):
                                nc.tensor.matmul(
                                    st[:, s, w:],
                                    qk_t[kslot][j // 4][
                                        pbase : pbase + D,
                                        (j % 4) * 128 : (j % 4 + 1) * 128,
                                    ],
                                    qk_t[qslot][jq][pbase : pbase + D, w:],
                                    start=True,
                                    stop=not diag,
                                    tile_position=(pbase, 0),
                                )
                            return st

                        def mask_pair(j, st):
                            if j >= 4 * jq:
                                w = (j - 4 * jq) * 128
                                for s in range(2):
                                    nc.tensor.matmul(
                                        st[:, s, w : w + 128],
                                        negI,
                                        triM,
                                        start=False,
                                        stop=True,
                                    )

                        def exp_block(j, st):
                            w = max(0, (j - 4 * jq) * 128)
                            est = attp.tile(
                                [128, 2, 512], bf16, tag="est", name="est"
                            )
                            nc.scalar.activation(
                                est[:, :, w:], st[:, :, w:], EXP, scale=SCALE
                            )
                            return est

                        def pv_block(j, state):
                            w = max(0, (j - 4 * jq) * 128)
                            for s, h in ((0, h0), (1, h1)):
                                nc.tensor.matmul(
                                    state["y2ps"][:, s, w:],
                                    v_t[j][:, h, :],
                                    state["ests"][j][:, s, w:],
                                    start=(j == 0),
                                    stop=(j == njt - 1),
                                )

                        def batch(jb, state=state):
                            def go():
                                if jb == 0:
                                    state["y2ps"] = ps_y.tile(
                                        [D + 1, 2, 512], f32, name="y2ps"
                                    )
                                stA = s_pair(jb, state)
                                stB = s_pair(jb + 1, state)
                                mask_pair(jb, stA)
                                mask_pair(jb + 1, stB)
                                state["ests"][jb] = exp_block(jb, stA)
                                state["ests"][jb + 1] = exp_block(jb + 1, stB)
                                if jb >= 2:
                                    pv_block(jb - 2, state)
                                    pv_block(jb - 1, state)

                            return go

                        def tail(state=state, hp=hp):
                            def go():
                                pv_block(njt - 2, state)
                                pv_block(njt - 1, state)
                                # epi part 1: denominator rows -> f32r sbuf
                                rsb = rsb_slots[(2 * jq + hp) % 2]
                                nc.vector.tensor_copy(
                                    rsb[D : D + 1, :, :],
                                    state["y2ps"][D : D + 1, :, :],
                                )

                            return go

                        def epi2(state=state, hp=hp):
                            def go():
                                # K=128 broadcast (full-array mode) -> DVE
                                # reciprocal -> psum-by-sbuf normalize.
                                rsb = rsb_slots[(2 * jq + hp) % 2]
                                y2ps = state["y2ps"]
                                rb_sb = attsmall.tile([D, 2, 512], f32, tag="rbs")
                                for s in range(2):
                                    rb_ps = ps_share.tile(
                                        [D, 512], f32, tag="share", name=f"rb{s}"
                                    )
                                    nc.tensor.matmul(
                                        rb_ps,
                                        ones_sb[:, :],
                                        rsb[:, s, :],
                                        start=True,
                                        stop=True,
                                    )
                                    nc.vector.reciprocal_approx_fast(
                                        rb_sb[:, s, :], rb_ps
                                    )
                                nc.vector.tensor_mul(
                                    y2_t[jq][0:D, hp, :],
                                    y2ps[0:D, 0, :],
                                    rb_sb[:, 0, :],
                                )
                                y_lo = attsmall.tile([D, 512], bf16, tag="ylo")
                                nc.vector.tensor_mul(
                                    y_lo, y2ps[0:D, 1, :], rb_sb[:, 1, :]
                                )
                                nc.gpsimd.dma_start(
                                    y2_t[jq][D:128, hp, :], y_lo
                                )

                            return go

                        for jb in range(0, njt, 2):
                            chunks.append(batch(jb))
                        chunks.append(tail())
                        chunks.append(epi2())

                    for hp in range(2):
                        build_hp(hp)
                    return chunks

                def weave(chunks, fillers):
                    n, m = len(chunks), len(fillers)
                    fi = 0
                    for i, ch in enumerate(chunks):
                        ch()
                        want = (i + 1) * m // n
                        while fi < want:
                            fillers[fi]()
                            fi += 1

                # DMA order matters for the kernel head: the data the first
                # matmuls need (stage-0 x, wqk) goes first; wp/consts are
                # only needed tens of us in.
                stage_dma(0)
                nc.sync.dma_start(wqk_sb, wqk[:, :, :])
                nc.sync.dma_start(wv_sb, wv[:, :, :])
                nc.sync.dma_start(consts_sb, consts[:, :])
                nc.sync.dma_start(ones_sb, ones64[:, :])
                nc.sync.dma_start(wp_sb, wp[:, :, :])

                # serial head: stage 0 qkv
                for g in stage_groups(0):
                    g()
                # each attention row jq=tb is woven with the next stage's qkv
                # groups and the previous block's c_proj as PE filler
                for tb in range(4):
                    fillers = []
                    if tb < 3:
                        stage_dma(tb + 1)
                        fillers += stage_groups(tb + 1)
                    if tb >= 1:
                        fillers += [proj_tile(tb - 1, 4 * (tb - 1) + i)
                                    for i in range(4)]
                    weave(attention_chunks(tb), fillers)
                for i in range(4):
                    proj_tile(3, 12 + i)()

    nc.compile()
    return nc


def _get_nc():
    if "nc" not in _CACHE:
        _CACHE["nc"] = _build()
    return _CACHE["nc"]


def make_in_maps(x, w_attn, w_proj):
    import ml_dtypes

    bf16 = ml_dtypes.bfloat16
    x = np.asarray(x, np.float32)
    w_attn = np.asarray(w_attn, np.float32)
    w_proj = np.asarray(w_proj, np.float32)

    negI = np.zeros((128, 128), np.float32)
    np.fill_diagonal(negI, NEG)
    triM = np.tril(np.ones((128, 128), np.float32), -1)
    consts = np.concatenate([negI, triM], axis=1).astype(bf16)
    ones64 = np.ones((128, 64), np.float32)

    in_maps = []
    for c in range(N_CORES):
        b, hg = c // 4, c % 4
        hs = hg * HL * D  # 256 * hg
        xt = np.ascontiguousarray(x[b].T)  # [C, T]
        # [128, tb, KT, 512]: per-partition-contiguous per stage
        x_t = xt.reshape(KT, 128, 4, T // 4).transpose(1, 2, 0, 3)
        wq = w_attn[hs : hs + HL * D, :]
        wk = w_attn[C + hs : C + hs + HL * D, :]
        wqkt = np.concatenate([wq, wk], 0).T  # [C, 512]
        wqk_t = wqkt.reshape(KT, 128, 2 * HL * D).transpose(1, 0, 2)
        wvt = w_attn[2 * C + hs : 2 * C + hs + HL * D, :].T  # [C, 256]
        wv_t = wvt.reshape(KT, 128, HL * D).transpose(1, 0, 2)
        # head-pair stacked rows: [128, HL//2, C]; partition p of pair pr is
        # local feature pr*128 + p (head 2*pr dims then head 2*pr+1 dims)
        wp_t = (
            w_proj[:, hs : hs + HL * D].T.reshape(HL // 2, 128, C).transpose(1, 0, 2)
        )
        in_maps.append(
            {
                "x_in": np.ascontiguousarray(x_t).astype(bf16),
                "wqk": np.ascontiguousarray(wqk_t).astype(bf16),
                "wv": np.ascontiguousarray(wv_t).astype(bf16),
                "wp": np.ascontiguousarray(wp_t).astype(bf16),
                "consts": consts,
                "ones64": ones64,
            }
        )
    return in_maps


def run(in_maps, **kwargs):
    nc = _get_nc()
    return run_bass_kernel_spmd(nc, in_maps, core_ids=list(range(N_CORES)), **kwargs)


def combine(results):
    out = np.zeros((B, T, C), np.float64)
    for c in range(N_CORES):
        out[c // 4] += results[c]["out"].astype(np.float64)
    return out.astype(np.float32)


def kernel(x, w_attn, w_proj):
    res = run(make_in_maps(x, w_attn, w_proj))
    return combine(res.results)


# revision 22
# speedup vs baseline: 1.1615x; 1.0084x over previous
"""Causal self-attention on 8 Trainium2 NeuronCores.

Sharding (batch + head parallel): core c handles batch b = c // 4 and the
4 heads [hg*4, hg*4+4) where hg = c % 4.  Each core computes q/k/v from
column-sliced c_attn weights, full causal attention for its heads, and a
partial c_proj output from the matching row slice of w_proj; the host sums
the 4 partials per batch.

All matmuls run in bf16 (fp32 PSUM accumulate).  The two heads of a pair
are computed CONCURRENTLY in the PE array for the S = K^T Q matmuls via
row tiling (K=64 contraction each, tile_position rows 0-63 / 64-127); S
batches of two key blocks keep the PE in 64-row tiling mode back-to-back
so the mode-switch drain is paid once per batch.  The causal mask is
applied inside PSUM with a -30000*tril matmul accumulate, so the
S -> exp -> PV chain only touches PE and ACT.

Emission WEAVES attention chunks with independent PE filler work (the
next stage's qkv matmul groups and c_proj halves) so the ACT exp stream
- the second-busiest engine - runs from ~15us in, and the PE always has
independent work queued behind ACT-dependent chunks (no head-of-line
stalls, HAM clock-gate stays at 2.4 GHz).
"""

import sys

if "/opt/trn_rl_repo" not in sys.path:
    sys.path.insert(0, "/opt/trn_rl_repo")

import numpy as np

import concourse.mybir as mybir
from concourse import bacc
from concourse.bass_utils import run_bass_kernel_spmd
from concourse.tile import TileContext

B, T, C = 2, 2048, 1024
H, D = 16, 64
HL = 4  # heads per core
N_CORES = 8
KT = C // 128  # contraction tiles over the embedding dim
SCALE = 1.0 / 8.0  # 1/sqrt(D)
NEG = -30000.0

_CACHE = {}


def _build():
    f32 = mybir.dt.float32
    f32r = mybir.dt.float32r
    bf16 = mybir.dt.bfloat16
    nc = bacc.Bacc("TRN2", target_bir_lowering=False, debug=False, num_devices=N_CORES)

    x_in = nc.dram_tensor("x_in", [128, 4, KT, T // 4], bf16, kind="ExternalInput")
    wqk = nc.dram_tensor("wqk", [128, KT, 2 * HL * D], bf16, kind="ExternalInput")
    wv = nc.dram_tensor("wv", [128, KT, HL * D], bf16, kind="ExternalInput")
    wp = nc.dram_tensor("wp", [128, HL // 2, C], bf16, kind="ExternalInput")
    # consts: cols 0:128 = diag(-30000); cols 128:256 = tril(ones, -1)
    consts = nc.dram_tensor("consts", [128, 256], bf16, kind="ExternalInput")
    ones64 = nc.dram_tensor("ones64", [128, 64], f32r, kind="ExternalInput")
    out = nc.dram_tensor("out", [T, C], bf16, kind="ExternalOutput")

    EXP = mybir.ActivationFunctionType.Exp

    with TileContext(nc) as tc:
        with tc.tile_pool(name="persist", bufs=1) as persist:
            # q/k feature-major [d, t]: slot 0/1 = q head-pairs 0/1, 2/3 = k;
            # partitions 0-63 = even head dims, 64-127 = odd head dims.
            qk_t = [
                [persist.tile([128, 512], bf16, name=f"qk{s}_{tb}") for tb in range(4)]
                for s in range(4)
            ]
            # v token-major per 128-token tile; col D holds ones (denominator)
            v_t = [
                persist.tile([128, HL, D + 1], bf16, name=f"v{tt}") for tt in range(16)
            ]
            # head-pair stacked normalized y per 512-token block
            y2_t = [
                persist.tile([128, HL // 2, 512], bf16, name=f"y2{b_}")
                for b_ in range(4)
            ]
            wp_sb = persist.tile([128, HL // 2, C], bf16)
            consts_sb = persist.tile([128, 256], bf16)
            negI = consts_sb[:, 0:128]
            triM = consts_sb[:, 128:256]
            ones_sb = persist.tile([128, 64], f32r)
            # denominator staging rows: all partitions zero except row D, so
            # the broadcast matmul can contract K=128 (full-array mode, no
            # tiling-mode switch/drain on the PE)
            rsb_slots = [
                persist.tile([128, 2, 512], f32r, name=f"rsb{i}") for i in range(2)
            ]
            for r_ in rsb_slots:
                nc.gpsimd.memset(r_.bitcast(mybir.dt.uint32), 0)

            for tt in range(16):
                nc.gpsimd.memset(v_t[tt][:, :, D : D + 1], 1.0)

            with (
                tc.tile_pool(name="qkvp", bufs=1) as qkvp,
                tc.tile_pool(name="attp", bufs=6) as attp,
                tc.tile_pool(name="attsmall", bufs=2) as attsmall,
                tc.tile_pool(name="projp", bufs=2) as projp,
                tc.tile_pool(name="ps_st", bufs=2, space="PSUM") as ps_st,
                tc.tile_pool(name="ps_y", bufs=1, space="PSUM") as ps_y,
                tc.tile_pool(name="ps_share", bufs=2, space="PSUM") as ps_share,
            ):
                # quarter-length x buffers, double-buffered across stages
                x_q = [
                    qkvp.tile([128, KT, T // 4], bf16, name=f"x_q{i}") for i in range(2)
                ]
                wqk_sb = qkvp.tile([128, KT, 2 * HL * D], bf16)
                wv_sb = qkvp.tile([128, KT, HL * D], bf16)

                def stage_dma(tb):
                    # stage-major x layout: 8KB contiguous per partition ->
                    # few large DMA descriptors instead of ~1K small ones
                    nc.sync.dma_start(x_q[tb % 2], x_in[:, tb])

                def qk_group(tb, jt):
                    def go():
                        x_sb = x_q[tb % 2]
                        qk_ps = ps_share.tile(
                            [128, 512], f32, tag="share", name="qk_ps"
                        )
                        for kt in range(KT):
                            nc.tensor.matmul(
                                qk_ps,
                                wqk_sb[:, kt, jt * 128 : (jt + 1) * 128],
                                x_sb[:, kt, :],
                                start=(kt == 0),
                                stop=(kt == KT - 1),
                            )
                        nc.vector.tensor_copy(qk_t[jt][tb], qk_ps)

                    return go

                def v_group(tb, tt2):
                    def go():
                        x_sb = x_q[tb % 2]
                        tt = tb * 4 + tt2
                        v_ps = ps_share.tile(
                            [128, HL * D], f32, tag="share", name="v_ps"
                        )
                        for kt in range(KT):
                            nc.tensor.matmul(
                                v_ps,
                                x_sb[:, kt, tt2 * 128 : (tt2 + 1) * 128],
                                wv_sb[:, kt, :],
                                start=(kt == 0),
                                stop=(kt == KT - 1),
                            )
                        nc.vector.tensor_copy(
                            v_t[tt][:, :, 0:D],
                            v_ps.rearrange("p (h d) -> p h d", h=HL),
                        )

                    return go

                def stage_groups(tb):
                    # zipped so q and k/v land evenly through the row
                    return [qk_group(tb, 0), qk_group(tb, 1), v_group(tb, 0),
                            qk_group(tb, 2), v_group(tb, 1), qk_group(tb, 3),
                            v_group(tb, 2), v_group(tb, 3)]

                def proj_tile(blk, tt):
                    def go():
                        o_sb = projp.tile([128, C], bf16, name="o_sb")
                        off = (tt % 4) * 128
                        for cb in range(2):
                            o_ps = ps_share.tile(
                                [128, 512], f32, tag="share", name="o_ps"
                            )
                            for pr in range(2):
                                nc.tensor.matmul(
                                    o_ps,
                                    y2_t[blk][:, pr, off : off + 128],
                                    wp_sb[:, pr, cb * 512 : (cb + 1) * 512],
                                    start=(pr == 0),
                                    stop=(pr == 1),
                                )
                            # the last block runs after the exp stream ends:
                            # evacuate on the then-idle ACT instead of DVE
                            if blk == 3:
                                nc.scalar.copy(
                                    o_sb[:, cb * 512 : (cb + 1) * 512], o_ps
                                )
                            else:
                                nc.vector.tensor_copy(
                                    o_sb[:, cb * 512 : (cb + 1) * 512], o_ps
                                )
                        nc.sync.dma_start(out[tt * 128 : (tt + 1) * 128, :], o_sb)

                    return go

                def attention_chunks(jq):
                    njt = 4 * (jq + 1)
                    chunks = []

                    def build_hp(hp):
                        # separate scope per hp: closures run later (in the
                        # weave), so they must bind this hp's values now
                        h0, h1 = 2 * hp, 2 * hp + 1
                        kslot, qslot = 2 + hp, hp
                        state = {"ests": [None] * njt}

                        def s_pair(j, state):
                            w = max(0, (j - 4 * jq) * 128)
                            st = ps_st.tile([128, 2, 512], f32, name="st")
                            diag = j >= 4 * jq
                            for s, pbase in ((0, 0), (1, 64)):
                                nc.tensor.matmul(
                                    st[:, s, w:],
                                    qk_t[kslot][j // 4][
                                        pbase : pbase + D,
                                        (j % 4) * 128 : (j % 4 + 1) * 128,
                                    ],
                                    qk_t[qslot][jq][pbase : pbase + D, w:],
                                    start=True,
                                    stop=not diag,
                                    tile_position=(pbase, 0),
                                )
                            return st

                        def mask_pair(j, st):
                            if j >= 4 * jq:
                                w = (j - 4 * jq) * 128
                                for s in range(2):
                                    nc.tensor.matmul(
                                        st[:, s, w : w + 128],
                                        negI,
                                        triM,
                                        start=False,
                                        stop=True,
                                    )

                        def exp_block(j, st):
                            w = max(0, (j - 4 * jq) * 128)
                            est = attp.tile(
                                [128, 2, 512], bf16, tag="est", name="est"
                            )
                            nc.scalar.activation(
                                est[:, :, w:], st[:, :, w:], EXP, scale=SCALE
                            )
                            return est

                        def pv_block(j, state):
                            w = max(0, (j - 4 * jq) * 128)
                            for s, h in ((0, h0), (1, h1)):
                                nc.tensor.matmul(
                                    state["y2ps"][:, s, w:],
                                    v_t[j][:, h, :],
                                    state["ests"][j][:, s, w:],
                                    start=(j == 0),
                                    stop=(j == njt - 1),
                                )

                        def batch(jb, state=state):
                            def go():
                                if jb == 0:
                                    state["y2ps"] = ps_y.tile(
                                        [D + 1, 2, 512], f32, name="y2ps"
                                    )
                                stA = s_pair(jb, state)
                                stB = s_pair(jb + 1, state)
                                mask_pair(jb, stA)
                                mask_pair(jb + 1, stB)
                                state["ests"][jb] = exp_block(jb, stA)
                                state["ests"][jb + 1] = exp_block(jb + 1, stB)
                                if jb >= 2:
                                    pv_block(jb - 2, state)
                                    pv_block(jb - 1, state)

                            return go

                        def tail(state=state, hp=hp):
                            def go():
                                pv_block(njt - 2, state)
                                pv_block(njt - 1, state)
                                # epi part 1: denominator rows -> f32r sbuf
                                rsb = rsb_slots[(2 * jq + hp) % 2]
                                nc.vector.tensor_copy(
                                    rsb[D : D + 1, :, :],
                                    state["y2ps"][D : D + 1, :, :],
                                )

                            return go

                        def epi2(state=state, hp=hp):
                            def go():
                                # K=128 broadcast (full-array mode) -> DVE
                                # reciprocal -> psum-by-sbuf normalize.
                                rsb = rsb_slots[(2 * jq + hp) % 2]
                                y2ps = state["y2ps"]
                                rb_sb = attsmall.tile([D, 2, 512], f32, tag="rbs")
                                for s in range(2):
                                    rb_ps = ps_share.tile(
                                        [D, 512], f32, tag="share", name=f"rb{s}"
                                    )
                                    nc.tensor.matmul(
                                        rb_ps,
                                        ones_sb[:, :],
                                        rsb[:, s, :],
                                        start=True,
                                        stop=True,
                                    )
                                    nc.vector.reciprocal_approx_fast(
                                        rb_sb[:, s, :], rb_ps
                                    )
                                nc.vector.tensor_mul(
                                    y2_t[jq][0:D, hp, :],
                                    y2ps[0:D, 0, :],
                                    rb_sb[:, 0, :],
                                )
                                y_lo = attsmall.tile([D, 512], bf16, tag="ylo")
                                nc.vector.tensor_mul(
                                    y_lo, y2ps[0:D, 1, :], rb_sb[:, 1, :]
                                )
                                nc.gpsimd.dma_start(
                                    y2_t[jq][D:128, hp, :], y_lo
                                )

                            return go

                        for jb in range(0, njt, 2):
                            chunks.append(batch(jb))
                        chunks.append(tail())
                        chunks.append(epi2())

                    for hp in range(2):
                        build_hp(hp)
                    return chunks

                def weave(chunks, fillers):
                    n, m = len(chunks), len(fillers)
                    fi = 0
                    for i, ch in enumerate(chunks):
                        ch()
                        want = (i + 1) * m // n
                        while fi < want:
                            fillers[fi]()
                            fi += 1

                # DMA order matters for the kernel head: the data the first
                # matmuls need (stage-0 x, wqk) goes first; wp/consts are
                # only needed tens of us in.
                stage_dma(0)
                nc.sync.dma_start(wqk_sb, wqk[:, :, :])
                nc.sync.dma_start(wv_sb, wv[:, :, :])
                nc.sync.dma_start(consts_sb, consts[:, :])
                nc.sync.dma_start(ones_sb, ones64[:, :])
                nc.sync.dma_start(wp_sb, wp[:, :, :])

                # serial head: stage 0 qkv
                for g in stage_groups(0):
                    g()
                # each attention row jq=tb is woven with the next stage's qkv
                # groups and the previous block's c_proj as PE filler
                for tb in range(4):
                    fillers = []
                    if tb < 3:
                        stage_dma(tb + 1)
                        fillers += stage_groups(tb + 1)
                    if tb >= 1:
                        fillers += [proj_tile(tb - 1, 4 * (tb - 1) + i)
                                    for i in range(4)]
                    weave(attention_chunks(tb), fillers)
                for i in range(4):
                    proj_tile(3, 12 + i)()

    nc.compile()
    return nc


def _get_nc():
    if "nc" not in _CACHE:
        _CACHE["nc"] = _build()
    return _CACHE["nc"]


def make_in_maps(x, w_attn, w_proj):
    import ml_dtypes

    bf16 = ml_dtypes.bfloat16
    x = np.asarray(x, np.float32)
    w_attn = np.asarray(w_attn, np.float32)
    w_proj = np.asarray(w_proj, np.float32)

    negI = np.zeros((128, 128), np.float32)
    np.fill_diagonal(negI, NEG)
    triM = np.tril(np.ones((128, 128), np.float32), -1)
    consts = np.concatenate([negI, triM], axis=1).astype(bf16)
    ones64 = np.ones((128, 64), np.float32)

    in_maps = []
    for c in range(N_CORES):
        b, hg = c // 4, c % 4
        hs = hg * HL * D  # 256 * hg
        xt = np.ascontiguousarray(x[b].T)  # [C, T]
        # [128, tb, KT, 512]: per-partition-contiguous per stage
        x_t = xt.reshape(KT, 128, 4, T // 4).transpose(1, 2, 0, 3)
        wq = w_attn[hs : hs + HL * D, :]
        wk = w_attn[C + hs : C + hs + HL * D, :]
        wqkt = np.concatenate([wq, wk], 0).T  # [C, 512]
        wqk_t = wqkt.reshape(KT, 128, 2 * HL * D).transpose(1, 0, 2)
        wvt = w_attn[2 * C + hs : 2 * C + hs + HL * D, :].T  # [C, 256]
        wv_t = wvt.reshape(KT, 128, HL * D).transpose(1, 0, 2)
        # head-pair stacked rows: [128, HL//2, C]; partition p of pair pr is
        # local feature pr*128 + p (head 2*pr dims then head 2*pr+1 dims)
        wp_t = (
            w_proj[:, hs : hs + HL * D].T.reshape(HL // 2, 128, C).transpose(1, 0, 2)
        )
        in_maps.append(
            {
                "x_in": np.ascontiguousarray(x_t).astype(bf16),
                "wqk": np.ascontiguousarray(wqk_t).astype(bf16),
                "wv": np.ascontiguousarray(wv_t).astype(bf16),
                "wp": np.ascontiguousarray(wp_t).astype(bf16),
                "consts": consts,
                "ones64": ones64,
            }
        )
    return in_maps


def run(in_maps, **kwargs):
    nc = _get_nc()
    return run_bass_kernel_spmd(nc, in_maps, core_ids=list(range(N_CORES)), **kwargs)


def combine(results):
    out = np.zeros((B, T, C), np.float64)
    for c in range(N_CORES):
        out[c // 4] += results[c]["out"].astype(np.float64)
    return out.astype(np.float32)


def kernel(x, w_attn, w_proj):
    res = run(make_in_maps(x, w_attn, w_proj))
    return combine(res.results)


# revision 23
# speedup vs baseline: 1.1703x; 1.0076x over previous
"""Causal self-attention on 8 Trainium2 NeuronCores.

Sharding (batch + head parallel): core c handles batch b = c // 4 and the
4 heads [hg*4, hg*4+4) where hg = c % 4.  Each core computes q/k/v from
column-sliced c_attn weights, full causal attention for its heads, and a
partial c_proj output from the matching row slice of w_proj; the host sums
the 4 partials per batch.

All matmuls run in bf16 (fp32 PSUM accumulate).  The two heads of a pair
are computed CONCURRENTLY in the PE array for the S = K^T Q matmuls via
row tiling (K=64 contraction each, tile_position rows 0-63 / 64-127); S
batches of two key blocks keep the PE in 64-row tiling mode back-to-back
so the mode-switch drain is paid once per batch.  The causal mask is
applied inside PSUM with a -30000*tril matmul accumulate, so the
S -> exp -> PV chain only touches PE and ACT.

Emission WEAVES attention chunks with independent PE filler work (the
next stage's qkv matmul groups and c_proj halves) so the ACT exp stream
- the second-busiest engine - runs from ~15us in, and the PE always has
independent work queued behind ACT-dependent chunks (no head-of-line
stalls, HAM clock-gate stays at 2.4 GHz).
"""

import sys

if "/opt/trn_rl_repo" not in sys.path:
    sys.path.insert(0, "/opt/trn_rl_repo")

import numpy as np

import concourse.mybir as mybir
from concourse import bacc
from concourse.bass_utils import run_bass_kernel_spmd
from concourse.tile import TileContext

B, T, C = 2, 2048, 1024
H, D = 16, 64
HL = 4  # heads per core
N_CORES = 8
KT = C // 128  # contraction tiles over the embedding dim
SCALE = 1.0 / 8.0  # 1/sqrt(D)
NEG = -30000.0

_CACHE = {}


def _build():
    f32 = mybir.dt.float32
    f32r = mybir.dt.float32r
    bf16 = mybir.dt.bfloat16
    nc = bacc.Bacc("TRN2", target_bir_lowering=False, debug=False, num_devices=N_CORES)

    x_in = nc.dram_tensor("x_in", [128, 4, KT, T // 4], bf16, kind="ExternalInput")
    wqk = nc.dram_tensor("wqk", [128, KT, 2 * HL * D], bf16, kind="ExternalInput")
    wv = nc.dram_tensor("wv", [128, KT, HL * D], bf16, kind="ExternalInput")
    wp = nc.dram_tensor("wp", [128, HL // 2, C], bf16, kind="ExternalInput")
    # consts: cols 0:128 = diag(-30000); cols 128:256 = tril(ones, -1)
    consts = nc.dram_tensor("consts", [128, 256], bf16, kind="ExternalInput")
    ones64 = nc.dram_tensor("ones64", [128, 64], f32r, kind="ExternalInput")
    out = nc.dram_tensor("out", [T, C], bf16, kind="ExternalOutput")

    EXP = mybir.ActivationFunctionType.Exp

    with TileContext(nc) as tc:
        with tc.tile_pool(name="persist", bufs=1) as persist:
            # q/k feature-major [d, t]: slot 0/1 = q head-pairs 0/1, 2/3 = k;
            # partitions 0-63 = even head dims, 64-127 = odd head dims.
            qk_t = [
                [persist.tile([128, 512], bf16, name=f"qk{s}_{tb}") for tb in range(4)]
                for s in range(4)
            ]
            # v token-major per 128-token tile; col D holds ones (denominator)
            v_t = [
                persist.tile([128, HL, D + 1], bf16, name=f"v{tt}") for tt in range(16)
            ]
            # head-pair stacked normalized y per 512-token block
            y2_t = [
                persist.tile([128, HL // 2, 512], bf16, name=f"y2{b_}")
                for b_ in range(4)
            ]
            wp_sb = persist.tile([128, HL // 2, C], bf16)
            consts_sb = persist.tile([128, 256], bf16)
            negI = consts_sb[:, 0:128]
            triM = consts_sb[:, 128:256]
            ones_sb = persist.tile([128, 64], f32r)
            # denominator staging rows: all partitions zero except row D, so
            # the broadcast matmul can contract K=128 (full-array mode, no
            # tiling-mode switch/drain on the PE)
            rsb_slots = [
                persist.tile([128, 2, 512], f32r, name=f"rsb{i}") for i in range(2)
            ]
            for r_ in rsb_slots:
                nc.gpsimd.memset(r_.bitcast(mybir.dt.uint32), 0)

            for tt in range(16):
                nc.gpsimd.memset(v_t[tt][:, :, D : D + 1], 1.0)

            with (
                tc.tile_pool(name="qkvp", bufs=1) as qkvp,
                tc.tile_pool(name="attp", bufs=10) as attp,
                tc.tile_pool(name="attsmall", bufs=2) as attsmall,
                tc.tile_pool(name="projp", bufs=2) as projp,
                tc.tile_pool(name="ps_st", bufs=2, space="PSUM") as ps_st,
                tc.tile_pool(name="ps_y", bufs=1, space="PSUM") as ps_y,
                tc.tile_pool(name="ps_share", bufs=2, space="PSUM") as ps_share,
            ):
                # quarter-length x buffers, double-buffered across stages
                x_q = [
                    qkvp.tile([128, KT, T // 4], bf16, name=f"x_q{i}") for i in range(2)
                ]
                wqk_sb = qkvp.tile([128, KT, 2 * HL * D], bf16)
                wv_sb = qkvp.tile([128, KT, HL * D], bf16)

                def stage_dma(tb):
                    # stage-major x layout: 8KB contiguous per partition ->
                    # few large DMA descriptors instead of ~1K small ones
                    nc.sync.dma_start(x_q[tb % 2], x_in[:, tb])

                def qk_group(tb, jt):
                    def go():
                        x_sb = x_q[tb % 2]
                        qk_ps = ps_share.tile(
                            [128, 512], f32, tag="share", name="qk_ps"
                        )
                        for kt in range(KT):
                            nc.tensor.matmul(
                                qk_ps,
                                wqk_sb[:, kt, jt * 128 : (jt + 1) * 128],
                                x_sb[:, kt, :],
                                start=(kt == 0),
                                stop=(kt == KT - 1),
                            )
                        nc.vector.tensor_copy(qk_t[jt][tb], qk_ps)

                    return go

                def v_group(tb, tt2):
                    def go():
                        x_sb = x_q[tb % 2]
                        tt = tb * 4 + tt2
                        v_ps = ps_share.tile(
                            [128, HL * D], f32, tag="share", name="v_ps"
                        )
                        for kt in range(KT):
                            nc.tensor.matmul(
                                v_ps,
                                x_sb[:, kt, tt2 * 128 : (tt2 + 1) * 128],
                                wv_sb[:, kt, :],
                                start=(kt == 0),
                                stop=(kt == KT - 1),
                            )
                        nc.vector.tensor_copy(
                            v_t[tt][:, :, 0:D],
                            v_ps.rearrange("p (h d) -> p h d", h=HL),
                        )

                    return go

                def stage_groups(tb):
                    # zipped so q and k/v land evenly through the row
                    return [qk_group(tb, 0), qk_group(tb, 1), v_group(tb, 0),
                            qk_group(tb, 2), v_group(tb, 1), qk_group(tb, 3),
                            v_group(tb, 2), v_group(tb, 3)]

                def proj_tile(blk, tt):
                    def go():
                        o_sb = projp.tile([128, C], bf16, name="o_sb")
                        off = (tt % 4) * 128
                        for cb in range(2):
                            o_ps = ps_share.tile(
                                [128, 512], f32, tag="share", name="o_ps"
                            )
                            for pr in range(2):
                                nc.tensor.matmul(
                                    o_ps,
                                    y2_t[blk][:, pr, off : off + 128],
                                    wp_sb[:, pr, cb * 512 : (cb + 1) * 512],
                                    start=(pr == 0),
                                    stop=(pr == 1),
                                )
                            # the last block runs after the exp stream ends:
                            # evacuate on the then-idle ACT instead of DVE
                            if blk == 3:
                                nc.scalar.copy(
                                    o_sb[:, cb * 512 : (cb + 1) * 512], o_ps
                                )
                            else:
                                nc.vector.tensor_copy(
                                    o_sb[:, cb * 512 : (cb + 1) * 512], o_ps
                                )
                        nc.sync.dma_start(out[tt * 128 : (tt + 1) * 128, :], o_sb)

                    return go

                def attention_chunks(jq):
                    njt = 4 * (jq + 1)
                    chunks = []

                    def build_hp(hp):
                        # separate scope per hp: closures run later (in the
                        # weave), so they must bind this hp's values now
                        h0, h1 = 2 * hp, 2 * hp + 1
                        kslot, qslot = 2 + hp, hp
                        state = {"ests": [None] * njt}

                        def s_pair(j, state):
                            w = max(0, (j - 4 * jq) * 128)
                            st = ps_st.tile([128, 2, 512], f32, name="st")
                            diag = j >= 4 * jq
                            for s, pbase in ((0, 0), (1, 64)):
                                nc.tensor.matmul(
                                    st[:, s, w:],
                                    qk_t[kslot][j // 4][
                                        pbase : pbase + D,
                                        (j % 4) * 128 : (j % 4 + 1) * 128,
                                    ],
                                    qk_t[qslot][jq][pbase : pbase + D, w:],
                                    start=True,
                                    stop=not diag,
                                    tile_position=(pbase, 0),
                                )
                            return st

                        def mask_pair(j, st):
                            if j >= 4 * jq:
                                w = (j - 4 * jq) * 128
                                for s in range(2):
                                    nc.tensor.matmul(
                                        st[:, s, w : w + 128],
                                        negI,
                                        triM,
                                        start=False,
                                        stop=True,
                                    )

                        def exp_block(j, st):
                            w = max(0, (j - 4 * jq) * 128)
                            est = attp.tile(
                                [128, 2, 512], bf16, tag="est", name="est"
                            )
                            nc.scalar.activation(
                                est[:, :, w:], st[:, :, w:], EXP, scale=SCALE
                            )
                            return est

                        def pv_block(j, state):
                            w = max(0, (j - 4 * jq) * 128)
                            for s, h in ((0, h0), (1, h1)):
                                nc.tensor.matmul(
                                    state["y2ps"][:, s, w:],
                                    v_t[j][:, h, :],
                                    state["ests"][j][:, s, w:],
                                    start=(j == 0),
                                    stop=(j == njt - 1),
                                )

                        def batch(jb, state=state):
                            def go():
                                if jb == 0:
                                    state["y2ps"] = ps_y.tile(
                                        [D + 1, 2, 512], f32, name="y2ps"
                                    )
                                stA = s_pair(jb, state)
                                stB = s_pair(jb + 1, state)
                                mask_pair(jb, stA)
                                mask_pair(jb + 1, stB)
                                state["ests"][jb] = exp_block(jb, stA)
                                state["ests"][jb + 1] = exp_block(jb + 1, stB)
                                if jb >= 4:
                                    pv_block(jb - 4, state)
                                    pv_block(jb - 3, state)

                            return go

                        def tail(state=state, hp=hp):
                            def go():
                                for j_ in range(max(0, njt - 4), njt):
                                    pv_block(j_, state)
                                # epi part 1: denominator rows -> f32r sbuf
                                rsb = rsb_slots[(2 * jq + hp) % 2]
                                nc.vector.tensor_copy(
                                    rsb[D : D + 1, :, :],
                                    state["y2ps"][D : D + 1, :, :],
                                )

                            return go

                        def epi2(state=state, hp=hp):
                            def go():
                                # K=128 broadcast (full-array mode) -> DVE
                                # reciprocal -> psum-by-sbuf normalize.
                                rsb = rsb_slots[(2 * jq + hp) % 2]
                                y2ps = state["y2ps"]
                                rb_sb = attsmall.tile([D, 2, 512], f32, tag="rbs")
                                for s in range(2):
                                    rb_ps = ps_share.tile(
                                        [D, 512], f32, tag="share", name=f"rb{s}"
                                    )
                                    nc.tensor.matmul(
                                        rb_ps,
                                        ones_sb[:, :],
                                        rsb[:, s, :],
                                        start=True,
                                        stop=True,
                                    )
                                    nc.vector.reciprocal_approx_fast(
                                        rb_sb[:, s, :], rb_ps
                                    )
                                nc.vector.tensor_mul(
                                    y2_t[jq][0:D, hp, :],
                                    y2ps[0:D, 0, :],
                                    rb_sb[:, 0, :],
                                )
                                y_lo = attsmall.tile([D, 512], bf16, tag="ylo")
                                nc.vector.tensor_mul(
                                    y_lo, y2ps[0:D, 1, :], rb_sb[:, 1, :]
                                )
                                nc.gpsimd.dma_start(
                                    y2_t[jq][D:128, hp, :], y_lo
                                )

                            return go

                        for jb in range(0, njt, 2):
                            chunks.append(batch(jb))
                        chunks.append(tail())
                        chunks.append(epi2())

                    for hp in range(2):
                        build_hp(hp)
                    return chunks

                def weave(chunks, fillers):
                    n, m = len(chunks), len(fillers)
                    fi = 0
                    for i, ch in enumerate(chunks):
                        ch()
                        want = (i + 1) * m // n
                        while fi < want:
                            fillers[fi]()
                            fi += 1

                # DMA order matters for the kernel head: the data the first
                # matmuls need (stage-0 x, wqk) goes first; wp/consts are
                # only needed tens of us in.
                stage_dma(0)
                nc.sync.dma_start(wqk_sb, wqk[:, :, :])
                nc.sync.dma_start(wv_sb, wv[:, :, :])
                nc.sync.dma_start(consts_sb, consts[:, :])
                nc.sync.dma_start(ones_sb, ones64[:, :])
                nc.sync.dma_start(wp_sb, wp[:, :, :])

                # serial head: stage 0 qkv
                for g in stage_groups(0):
                    g()
                # each attention row jq=tb is woven with the next stage's qkv
                # groups and the previous block's c_proj as PE filler
                for tb in range(4):
                    fillers = []
                    if tb < 3:
                        stage_dma(tb + 1)
                        fillers += stage_groups(tb + 1)
                    if tb >= 1:
                        fillers += [proj_tile(tb - 1, 4 * (tb - 1) + i)
                                    for i in range(4)]
                    weave(attention_chunks(tb), fillers)
                for i in range(4):
                    proj_tile(3, 12 + i)()

    nc.compile()
    return nc


def _get_nc():
    if "nc" not in _CACHE:
        _CACHE["nc"] = _build()
    return _CACHE["nc"]


def make_in_maps(x, w_attn, w_proj):
    import ml_dtypes

    bf16 = ml_dtypes.bfloat16
    x = np.asarray(x, np.float32)
    w_attn = np.asarray(w_attn, np.float32)
    w_proj = np.asarray(w_proj, np.float32)

    negI = np.zeros((128, 128), np.float32)
    np.fill_diagonal(negI, NEG)
    triM = np.tril(np.ones((128, 128), np.float32), -1)
    consts = np.concatenate([negI, triM], axis=1).astype(bf16)
    ones64 = np.ones((128, 64), np.float32)

    in_maps = []
    for c in range(N_CORES):
        b, hg = c // 4, c % 4
        hs = hg * HL * D  # 256 * hg
        xt = np.ascontiguousarray(x[b].T)  # [C, T]
        # [128, tb, KT, 512]: per-partition-contiguous per stage
        x_t = xt.reshape(KT, 128, 4, T // 4).transpose(1, 2, 0, 3)
        wq = w_attn[hs : hs + HL * D, :]
        wk = w_attn[C + hs : C + hs + HL * D, :]
        wqkt = np.concatenate([wq, wk], 0).T  # [C, 512]
        wqk_t = wqkt.reshape(KT, 128, 2 * HL * D).transpose(1, 0, 2)
        wvt = w_attn[2 * C + hs : 2 * C + hs + HL * D, :].T  # [C, 256]
        wv_t = wvt.reshape(KT, 128, HL * D).transpose(1, 0, 2)
        # head-pair stacked rows: [128, HL//2, C]; partition p of pair pr is
        # local feature pr*128 + p (head 2*pr dims then head 2*pr+1 dims)
        wp_t = (
            w_proj[:, hs : hs + HL * D].T.reshape(HL // 2, 128, C).transpose(1, 0, 2)
        )
        in_maps.append(
            {
                "x_in": np.ascontiguousarray(x_t).astype(bf16),
                "wqk": np.ascontiguousarray(wqk_t).astype(bf16),
                "wv": np.ascontiguousarray(wv_t).astype(bf16),
                "wp": np.ascontiguousarray(wp_t).astype(bf16),
                "consts": consts,
                "ones64": ones64,
            }
        )
    return in_maps


def run(in_maps, **kwargs):
    nc = _get_nc()
    return run_bass_kernel_spmd(nc, in_maps, core_ids=list(range(N_CORES)), **kwargs)


def combine(results):
    out = np.zeros((B, T, C), np.float64)
    for c in range(N_CORES):
        out[c // 4] += results[c]["out"].astype(np.float64)
    return out.astype(np.float32)


def kernel(x, w_attn, w_proj):
    res = run(make_in_maps(x, w_attn, w_proj))
    return combine(res.results)


# revision 24
# speedup vs baseline: 1.1744x; 1.0036x over previous
"""Causal self-attention on 8 Trainium2 NeuronCores.

Sharding (batch + head parallel): core c handles batch b = c // 4 and the
4 heads [hg*4, hg*4+4) where hg = c % 4.  Each core computes q/k/v from
column-sliced c_attn weights, full causal attention for its heads, and a
partial c_proj output from the matching row slice of w_proj; the host sums
the 4 partials per batch.

All matmuls run in bf16 (fp32 PSUM accumulate).  The two heads of a pair
are computed CONCURRENTLY in the PE array for the S = K^T Q matmuls via
row tiling (K=64 contraction each, tile_position rows 0-63 / 64-127); S
batches of two key blocks keep the PE in 64-row tiling mode back-to-back
so the mode-switch drain is paid once per batch.  The causal mask is
applied inside PSUM with a -30000*tril matmul accumulate, so the
S -> exp -> PV chain only touches PE and ACT.

Emission WEAVES attention chunks with independent PE filler work (the
next stage's qkv matmul groups and c_proj halves) so the ACT exp stream
- the second-busiest engine - runs from ~15us in, and the PE always has
independent work queued behind ACT-dependent chunks (no head-of-line
stalls, HAM clock-gate stays at 2.4 GHz).
"""

import sys

if "/opt/trn_rl_repo" not in sys.path:
    sys.path.insert(0, "/opt/trn_rl_repo")

import numpy as np

import concourse.mybir as mybir
from concourse import bacc
from concourse.bass_utils import run_bass_kernel_spmd
from concourse.tile import TileContext

B, T, C = 2, 2048, 1024
H, D = 16, 64
HL = 4  # heads per core
N_CORES = 8
KT = C // 128  # contraction tiles over the embedding dim
SCALE = 1.0 / 8.0  # 1/sqrt(D)
NEG = -30000.0

_CACHE = {}


def _build():
    f32 = mybir.dt.float32
    f32r = mybir.dt.float32r
    bf16 = mybir.dt.bfloat16
    nc = bacc.Bacc("TRN2", target_bir_lowering=False, debug=False, num_devices=N_CORES)

    x_in = nc.dram_tensor("x_in", [128, 4, KT, T // 4], bf16, kind="ExternalInput")
    wqk = nc.dram_tensor("wqk", [128, KT, 2 * HL * D], bf16, kind="ExternalInput")
    wv = nc.dram_tensor("wv", [128, KT, HL * D], bf16, kind="ExternalInput")
    wp = nc.dram_tensor("wp", [128, HL // 2, C], bf16, kind="ExternalInput")
    # consts: cols 0:128 = diag(-30000); cols 128:256 = tril(ones, -1)
    consts = nc.dram_tensor("consts", [128, 256], bf16, kind="ExternalInput")
    ones64 = nc.dram_tensor("ones64", [128, 64], f32r, kind="ExternalInput")
    out = nc.dram_tensor("out", [T, C], bf16, kind="ExternalOutput")

    EXP = mybir.ActivationFunctionType.Exp

    with TileContext(nc) as tc:
        with tc.tile_pool(name="persist", bufs=1) as persist:
            # q/k feature-major [d, t]: slot 0/1 = q head-pairs 0/1, 2/3 = k;
            # partitions 0-63 = even head dims, 64-127 = odd head dims.
            qk_t = [
                [persist.tile([128, 512], bf16, name=f"qk{s}_{tb}") for tb in range(4)]
                for s in range(4)
            ]
            # v token-major per 128-token tile; col D holds ones (denominator)
            v_t = [
                persist.tile([128, HL, D + 1], bf16, name=f"v{tt}") for tt in range(16)
            ]
            # head-pair stacked normalized y per 512-token block
            y2_t = [
                persist.tile([128, HL // 2, 512], bf16, name=f"y2{b_}")
                for b_ in range(4)
            ]
            wp_sb = persist.tile([128, HL // 2, C], bf16)
            consts_sb = persist.tile([128, 256], bf16)
            negI = consts_sb[:, 0:128]
            triM = consts_sb[:, 128:256]
            ones_sb = persist.tile([128, 64], f32r)
            # denominator staging rows: all partitions zero except row D, so
            # the broadcast matmul can contract K=128 (full-array mode, no
            # tiling-mode switch/drain on the PE)
            rsb_slots = [
                persist.tile([128, 2, 512], f32r, name=f"rsb{i}") for i in range(2)
            ]
            for r_ in rsb_slots:
                nc.gpsimd.memset(r_.bitcast(mybir.dt.uint32), 0)

            for tt in range(16):
                nc.gpsimd.memset(v_t[tt][:, :, D : D + 1], 1.0)

            with (
                tc.tile_pool(name="qkvp", bufs=1) as qkvp,
                tc.tile_pool(name="attp", bufs=10) as attp,
                tc.tile_pool(name="attsmall", bufs=3) as attsmall,
                tc.tile_pool(name="projp", bufs=3) as projp,
                tc.tile_pool(name="ps_st", bufs=2, space="PSUM") as ps_st,
                tc.tile_pool(name="ps_y", bufs=1, space="PSUM") as ps_y,
                tc.tile_pool(name="ps_share", bufs=2, space="PSUM") as ps_share,
            ):
                # quarter-length x buffers, double-buffered across stages
                x_q = [
                    qkvp.tile([128, KT, T // 4], bf16, name=f"x_q{i}") for i in range(2)
                ]
                wqk_sb = qkvp.tile([128, KT, 2 * HL * D], bf16)
                wv_sb = qkvp.tile([128, KT, HL * D], bf16)

                def stage_dma(tb):
                    # stage-major x layout: 8KB contiguous per partition ->
                    # few large DMA descriptors instead of ~1K small ones
                    nc.sync.dma_start(x_q[tb % 2], x_in[:, tb])

                def qk_group(tb, jt):
                    def go():
                        x_sb = x_q[tb % 2]
                        qk_ps = ps_share.tile(
                            [128, 512], f32, tag="share", name="qk_ps"
                        )
                        for kt in range(KT):
                            nc.tensor.matmul(
                                qk_ps,
                                wqk_sb[:, kt, jt * 128 : (jt + 1) * 128],
                                x_sb[:, kt, :],
                                start=(kt == 0),
                                stop=(kt == KT - 1),
                            )
                        nc.vector.tensor_copy(qk_t[jt][tb], qk_ps)

                    return go

                def v_group(tb, tt2):
                    def go():
                        x_sb = x_q[tb % 2]
                        tt = tb * 4 + tt2
                        v_ps = ps_share.tile(
                            [128, HL * D], f32, tag="share", name="v_ps"
                        )
                        for kt in range(KT):
                            nc.tensor.matmul(
                                v_ps,
                                x_sb[:, kt, tt2 * 128 : (tt2 + 1) * 128],
                                wv_sb[:, kt, :],
                                start=(kt == 0),
                                stop=(kt == KT - 1),
                            )
                        nc.vector.tensor_copy(
                            v_t[tt][:, :, 0:D],
                            v_ps.rearrange("p (h d) -> p h d", h=HL),
                        )

                    return go

                def stage_groups(tb):
                    # zipped so q and k/v land evenly through the row
                    return [qk_group(tb, 0), qk_group(tb, 1), v_group(tb, 0),
                            qk_group(tb, 2), v_group(tb, 1), qk_group(tb, 3),
                            v_group(tb, 2), v_group(tb, 3)]

                def proj_tile(blk, tt):
                    def go():
                        o_sb = projp.tile([128, C], bf16, name="o_sb")
                        off = (tt % 4) * 128
                        for cb in range(2):
                            o_ps = ps_share.tile(
                                [128, 512], f32, tag="share", name="o_ps"
                            )
                            for pr in range(2):
                                nc.tensor.matmul(
                                    o_ps,
                                    y2_t[blk][:, pr, off : off + 128],
                                    wp_sb[:, pr, cb * 512 : (cb + 1) * 512],
                                    start=(pr == 0),
                                    stop=(pr == 1),
                                )
                            # the last block runs after the exp stream ends:
                            # evacuate on the then-idle ACT instead of DVE
                            if blk == 3:
                                nc.scalar.copy(
                                    o_sb[:, cb * 512 : (cb + 1) * 512], o_ps
                                )
                            else:
                                nc.vector.tensor_copy(
                                    o_sb[:, cb * 512 : (cb + 1) * 512], o_ps
                                )
                        nc.sync.dma_start(out[tt * 128 : (tt + 1) * 128, :], o_sb)

                    return go

                def attention_chunks(jq):
                    njt = 4 * (jq + 1)
                    chunks = []

                    def build_hp(hp):
                        # separate scope per hp: closures run later (in the
                        # weave), so they must bind this hp's values now
                        h0, h1 = 2 * hp, 2 * hp + 1
                        kslot, qslot = 2 + hp, hp
                        state = {"ests": [None] * njt}

                        def s_pair(j, state):
                            w = max(0, (j - 4 * jq) * 128)
                            st = ps_st.tile([128, 2, 512], f32, name="st")
                            diag = j >= 4 * jq
                            for s, pbase in ((0, 0), (1, 64)):
                                nc.tensor.matmul(
                                    st[:, s, w:],
                                    qk_t[kslot][j // 4][
                                        pbase : pbase + D,
                                        (j % 4) * 128 : (j % 4 + 1) * 128,
                                    ],
                                    qk_t[qslot][jq][pbase : pbase + D, w:],
                                    start=True,
                                    stop=not diag,
                                    tile_position=(pbase, 0),
                                )
                            return st

                        def mask_pair(j, st):
                            if j >= 4 * jq:
                                w = (j - 4 * jq) * 128
                                for s in range(2):
                                    nc.tensor.matmul(
                                        st[:, s, w : w + 128],
                                        negI,
                                        triM,
                                        start=False,
                                        stop=True,
                                    )

                        def exp_block(j, st):
                            w = max(0, (j - 4 * jq) * 128)
                            est = attp.tile(
                                [128, 2, 512], bf16, tag="est", name="est"
                            )
                            nc.scalar.activation(
                                est[:, :, w:], st[:, :, w:], EXP, scale=SCALE
                            )
                            return est

                        def pv_block(j, state):
                            w = max(0, (j - 4 * jq) * 128)
                            for s, h in ((0, h0), (1, h1)):
                                nc.tensor.matmul(
                                    state["y2ps"][:, s, w:],
                                    v_t[j][:, h, :],
                                    state["ests"][j][:, s, w:],
                                    start=(j == 0),
                                    stop=(j == njt - 1),
                                )

                        def batch(jb, state=state):
                            def go():
                                if jb == 0:
                                    state["y2ps"] = ps_y.tile(
                                        [D + 1, 2, 512], f32, name="y2ps"
                                    )
                                stA = s_pair(jb, state)
                                stB = s_pair(jb + 1, state)
                                mask_pair(jb, stA)
                                mask_pair(jb + 1, stB)
                                state["ests"][jb] = exp_block(jb, stA)
                                state["ests"][jb + 1] = exp_block(jb + 1, stB)
                                if jb >= 4:
                                    pv_block(jb - 4, state)
                                    pv_block(jb - 3, state)

                            return go

                        def tail(state=state, hp=hp):
                            def go():
                                for j_ in range(max(0, njt - 4), njt):
                                    pv_block(j_, state)
                                # epi part 1: denominator rows -> f32r sbuf
                                rsb = rsb_slots[(2 * jq + hp) % 2]
                                nc.vector.tensor_copy(
                                    rsb[D : D + 1, :, :],
                                    state["y2ps"][D : D + 1, :, :],
                                )

                            return go

                        def epi2(state=state, hp=hp):
                            def go():
                                # K=128 broadcast (full-array mode) -> DVE
                                # reciprocal -> psum-by-sbuf normalize.
                                rsb = rsb_slots[(2 * jq + hp) % 2]
                                y2ps = state["y2ps"]
                                rb_sb = attsmall.tile([D, 2, 512], f32, tag="rbs")
                                for s in range(2):
                                    rb_ps = ps_share.tile(
                                        [D, 512], f32, tag="share", name=f"rb{s}"
                                    )
                                    nc.tensor.matmul(
                                        rb_ps,
                                        ones_sb[:, :],
                                        rsb[:, s, :],
                                        start=True,
                                        stop=True,
                                    )
                                    nc.vector.reciprocal_approx_fast(
                                        rb_sb[:, s, :], rb_ps
                                    )
                                nc.vector.tensor_mul(
                                    y2_t[jq][0:D, hp, :],
                                    y2ps[0:D, 0, :],
                                    rb_sb[:, 0, :],
                                )
                                y_lo = attsmall.tile([D, 512], bf16, tag="ylo")
                                nc.vector.tensor_mul(
                                    y_lo, y2ps[0:D, 1, :], rb_sb[:, 1, :]
                                )
                                nc.gpsimd.dma_start(
                                    y2_t[jq][D:128, hp, :], y_lo
                                )

                            return go

                        for jb in range(0, njt, 2):
                            chunks.append(batch(jb))
                        chunks.append(tail())
                        chunks.append(epi2())

                    for hp in range(2):
                        build_hp(hp)
                    return chunks

                def weave(chunks, fillers):
                    n, m = len(chunks), len(fillers)
                    fi = 0
                    for i, ch in enumerate(chunks):
                        ch()
                        want = (i + 1) * m // n
                        while fi < want:
                            fillers[fi]()
                            fi += 1

                # DMA order matters for the kernel head: the data the first
                # matmuls need (stage-0 x, wqk) goes first; wp/consts are
                # only needed tens of us in.
                stage_dma(0)
                nc.sync.dma_start(wqk_sb, wqk[:, :, :])
                nc.sync.dma_start(wv_sb, wv[:, :, :])
                nc.sync.dma_start(consts_sb, consts[:, :])
                nc.sync.dma_start(ones_sb, ones64[:, :])
                nc.sync.dma_start(wp_sb, wp[:, :, :])

                # serial head: stage 0 qkv
                for g in stage_groups(0):
                    g()
                # each attention row jq=tb is woven with the next stage's qkv
                # groups and the previous block's c_proj as PE filler
                for tb in range(4):
                    fillers = []
                    if tb < 3:
                        stage_dma(tb + 1)
                        fillers += stage_groups(tb + 1)
                    if tb >= 1:
                        fillers += [proj_tile(tb - 1, 4 * (tb - 1) + i)
                                    for i in range(4)]
                    weave(attention_chunks(tb), fillers)
                for i in range(4):
                    proj_tile(3, 12 + i)()

    nc.compile()
    return nc


def _get_nc():
    if "nc" not in _CACHE:
        _CACHE["nc"] = _build()
    return _CACHE["nc"]


def make_in_maps(x, w_attn, w_proj):
    import ml_dtypes

    bf16 = ml_dtypes.bfloat16
    x = np.asarray(x, np.float32)
    w_attn = np.asarray(w_attn, np.float32)
    w_proj = np.asarray(w_proj, np.float32)

    negI = np.zeros((128, 128), np.float32)
    np.fill_diagonal(negI, NEG)
    triM = np.tril(np.ones((128, 128), np.float32), -1)
    consts = np.concatenate([negI, triM], axis=1).astype(bf16)
    ones64 = np.ones((128, 64), np.float32)

    in_maps = []
    for c in range(N_CORES):
        b, hg = c // 4, c % 4
        hs = hg * HL * D  # 256 * hg
        xt = np.ascontiguousarray(x[b].T)  # [C, T]
        # [128, tb, KT, 512]: per-partition-contiguous per stage
        x_t = xt.reshape(KT, 128, 4, T // 4).transpose(1, 2, 0, 3)
        wq = w_attn[hs : hs + HL * D, :]
        wk = w_attn[C + hs : C + hs + HL * D, :]
        wqkt = np.concatenate([wq, wk], 0).T  # [C, 512]
        wqk_t = wqkt.reshape(KT, 128, 2 * HL * D).transpose(1, 0, 2)
        wvt = w_attn[2 * C + hs : 2 * C + hs + HL * D, :].T  # [C, 256]
        wv_t = wvt.reshape(KT, 128, HL * D).transpose(1, 0, 2)
        # head-pair stacked rows: [128, HL//2, C]; partition p of pair pr is
        # local feature pr*128 + p (head 2*pr dims then head 2*pr+1 dims)
        wp_t = (
            w_proj[:, hs : hs + HL * D].T.reshape(HL // 2, 128, C).transpose(1, 0, 2)
        )
        in_maps.append(
            {
                "x_in": np.ascontiguousarray(x_t).astype(bf16),
                "wqk": np.ascontiguousarray(wqk_t).astype(bf16),
                "wv": np.ascontiguousarray(wv_t).astype(bf16),
                "wp": np.ascontiguousarray(wp_t).astype(bf16),
                "consts": consts,
                "ones64": ones64,
            }
        )
    return in_maps


def run(in_maps, **kwargs):
    nc = _get_nc()
    return run_bass_kernel_spmd(nc, in_maps, core_ids=list(range(N_CORES)), **kwargs)


def combine(results):
    out = np.zeros((B, T, C), np.float64)
    for c in range(N_CORES):
        out[c // 4] += results[c]["out"].astype(np.float64)
    return out.astype(np.float32)


def kernel(x, w_attn, w_proj):
    res = run(make_in_maps(x, w_attn, w_proj))
    return combine(res.results)
